# revision 68
# baseline (speedup 1.0000x reference)
"""Attention-LSTM captioning model on 8 trn2 cores (8-way tensor parallel).

Gate/itr/vocab output dims sharded across cores (full B=64 per core);
attention batch-sharded (8 batches/core, selected via per-core one-hot bsel
data, since the SPMD program is identical on every core). Activations are
transposed [feature, batch]. Per step: AllGather(att_resT + piggybacked
log-softmax stats), AllGather(nh chunk). Sigmoid(x) is computed as
(tanh(x/2)+1)/2 so the step only needs the {tanh, exp} ACT table; the hidden
state is stored as 2*h with h-consuming weights pre-halved on the host.

Per-step schedule fills both AllGather stall windows: the logit matmul for
step t-1, the gates matmul for step t, per-step log-softmax stats (bf16-
rounded max trick so the stats collapse to 2 bf16 scalars per row riding the
next AG_A), and the final logZ-subtract + output DMA for step t-2.
"""
import numpy as np
import ml_dtypes

import concourse.bacc as bacc
import concourse.mybir as mybir
import concourse.tile as tile
from concourse.ap import AP
from concourse.bass_utils import run_bass_kernel_spmd

BF16_NP = ml_dtypes.bfloat16
FP32 = mybir.dt.float32
BF16 = mybir.dt.bfloat16
AF = mybir.ActivationFunctionType
ALU = mybir.AluOpType
AX = mybir.AxisListType

B, T, R, H, F, E, L, V1 = 64, 20, 1024, 512, 2048, 300, 196, 12001
NC = 8
BMY = B // NC
GC = R // NC              # 128
NGATE = 5 * GC            # 640
VP = 1504
LP = 208
NG = LP // 16             # 13
EP = 384
HCN = H // 128            # 4
FCN = F // 128            # 16
RCN = R // 128            # 8
NBL = BMY * L             # 1568


def _bf(x):
    return np.ascontiguousarray(np.asarray(x, dtype=np.float32)).astype(BF16_NP)


def bcast_free(ap, n):
    """Append a step-0 free dim of size n to an AP (broadcast)."""
    return AP(ap.tensor, ap.offset, list(ap.ap) + [[0, n]])


def host_prep(inputs):
    seq = np.asarray(inputs["seq"])
    att = np.asarray(inputs["att_feats"], dtype=np.float32)
    embed_w = np.asarray(inputs["embed_w"], dtype=np.float32)
    ctx2att_w = np.asarray(inputs["ctx2att_w"], dtype=np.float32)
    ctx2att_b = np.asarray(inputs["ctx2att_b"], dtype=np.float32)
    h2att_w = np.asarray(inputs["h2att_w"], dtype=np.float32)
    h2att_b = np.asarray(inputs["h2att_b"], dtype=np.float32)
    alpha_w = np.asarray(inputs["alpha_w"], dtype=np.float32)
    i2h_w = np.asarray(inputs["i2h_w"], dtype=np.float32)
    i2h_b = np.asarray(inputs["i2h_b"], dtype=np.float32)
    h2h_w = np.asarray(inputs["h2h_w"], dtype=np.float32)
    h2h_b = np.asarray(inputs["h2h_b"], dtype=np.float32)
    a2c_w = np.asarray(inputs["a2c_w"], dtype=np.float32)
    a2c_b = np.asarray(inputs["a2c_b"], dtype=np.float32)
    logit_w = np.asarray(inputs["logit_w"], dtype=np.float32)
    logit_b = np.asarray(inputs["logit_b"], dtype=np.float32)

    xt = embed_w[seq]                                    # [B, T, E]
    xtT = np.zeros((EP, T * B), dtype=np.float32)
    xtT[:E] = xt.transpose(2, 1, 0).reshape(E, T * B)
    xtT[E] = 1.0
    xtT = _bf(xtT)
    bias_gate = i2h_b + h2h_b

    in_maps = []
    for c in range(NC):
        m = {"xtT": xtT}
        grows = np.concatenate([np.arange(gg * R + c * GC, gg * R + (c + 1) * GC)
                                for gg in range(5)])
        i2hT = np.zeros((EP, NGATE), dtype=np.float32)
        i2hT[:E] = i2h_w[grows, :].T
        i2hT[E] = bias_gate[grows]
        # fold the a2c bias into the itr-part gate bias (itr = sums + ctx)
        arows_b = np.concatenate([np.arange(c * GC, (c + 1) * GC),
                                  np.arange(R + c * GC, R + (c + 1) * GC)])
        i2hT[E, 384:640] += a2c_b[arows_b]
        m["i2hT"] = _bf(i2hT)
        m["h2hT"] = _bf(h2h_w[grows, :].T * 0.5)
        m["h2attT"] = _bf(h2att_w.T * 0.5)
        m["ctxT"] = _bf(ctx2att_w.T)
        # h2att_bias folded in: dot = tanh(p_att + h@h2attT) with both
        # biases additive per-h element
        m["ctx_bias"] = _bf((ctx2att_b + h2att_b)[None, :])
        amy = att[c * BMY:(c + 1) * BMY]                 # [8, L, F]
        m["attT"] = _bf(amy.transpose(2, 0, 1).reshape(F, NBL))
        alb = np.zeros((NG * 16, 8, F), dtype=np.float32)
        alb[:L] = amy.transpose(1, 0, 2)
        m["att_lb"] = _bf(alb.reshape(NG * 128, F))
        ac = np.zeros((HCN * 128, 64), dtype=np.float32)
        for b in range(BMY):
            ac[:, b * 8 + b] = alpha_w[0]
        m["alpha_cols"] = _bf(ac)
        arows = np.concatenate([np.arange(c * GC, (c + 1) * GC),
                                np.arange(R + c * GC, R + (c + 1) * GC)])
        m["a2cT"] = _bf(a2c_w[arows, :].T)
        vrows = np.arange(c * VP, (c + 1) * VP)
        lw = np.zeros((R, VP), dtype=np.float32)
        lb = np.full((1, VP), -1e30, dtype=np.float32)
        valid = vrows < V1
        lw[:, valid] = logit_w[vrows[valid], :].T * 0.5
        lb[0, valid] = logit_b[vrows[valid]]
        m["logitT"] = _bf(lw)
        m["logit_bias"] = lb
        m["ident"] = _bf(np.eye(128))
        bsel = np.zeros((B, BMY), dtype=np.float32)
        for j in range(BMY):
            bsel[c * BMY + j, j] = 1.0
        m["bsel"] = _bf(bsel)
        in_maps.append(m)
    return in_maps


def build(t_steps=T, probes=(), reps=1, no_cc=False):
    nc = bacc.Bacc("TRN2", target_bir_lowering=False, debug=False,
                   num_devices=NC)
    probes = set(probes)
    RG = [list(range(NC))]
    AGW = 130                 # agA payload: 128 arT cols + 2 stat cols

    def din(name, shape, dt=BF16):
        return nc.dram_tensor(name, shape, dt, kind="ExternalInput")

    xtT_d = din("xtT", [EP, T * B])
    i2hT_d = din("i2hT", [EP, NGATE])
    h2hT_d = din("h2hT", [R, NGATE])
    h2attT_d = din("h2attT", [R, H])
    ctxT_d = din("ctxT", [F, H])
    ctx_b_d = din("ctx_bias", [1, H])
    attT_d = din("attT", [F, NBL])
    att_lb_d = din("att_lb", [NG * 128, F])
    alpha_d = din("alpha_cols", [HCN * 128, 64])
    a2cT_d = din("a2cT", [F, 256])
    logitT_d = din("logitT", [R, VP])
    logit_b_d = din("logit_bias", [1, VP], FP32)
    ident_d = din("ident", [128, 128])
    bsel_d = din("bsel", [B, BMY])

    out_d = nc.dram_tensor("logp", [t_steps * B, VP], FP32,
                           kind="ExternalOutput")
    agA_out_r = [[nc.dram_tensor(f"agA_out_{rp}_{t}", [NC * 128, AGW], BF16,
                                 addr_space="Shared") for t in range(t_steps)]
                 for rp in range(reps)]
    agH_out_r = [[nc.dram_tensor(f"agH_out_{rp}_{t}", [R, B], BF16,
                                 addr_space="Shared") for t in range(t_steps)]
                 for rp in range(reps)]
    agS_out_r = [nc.dram_tensor(f"agS_out_{rp}", [NC * 64, 4], BF16,
                                addr_space="Shared") for rp in range(reps)]

    with tile.TileContext(nc) as tc:
        with (
            tc.tile_pool(name="wpool", bufs=1) as wpool,
            tc.tile_pool(name="hpool", bufs=4) as hpool,
            tc.tile_pool(name="psum", bufs=1, space="PSUM") as psum,
            tc.tile_pool(name="dram", bufs=4, space="DRAM") as dpool,
        ):
            def probe_(name, src_ap, shape, dt):
                pd = nc.dram_tensor(f"probe_{name}", list(shape), dt,
                                    kind="ExternalOutput")
                nc.sync.dma_start(out=pd[:], in_=src_ap)

            def load_chunks(pool, dram, cols, n, tag, dt=BF16):
                ts = []
                for i in range(n):
                    t_ = pool.tile([128, cols], dt, tag=f"{tag}{i}",
                                   name=f"{tag}{i}")
                    nc.sync.dma_start(out=t_[:],
                                      in_=dram[i * 128:(i + 1) * 128, :])
                    ts.append(t_)
                return ts

            logitT_s = load_chunks(wpool, logitT_d, VP, RCN, "logitT")
            logit_b_s = wpool.tile([64, VP], FP32, tag="logitb",
                                   name="logitb")
            _lb_src = AP(logit_b_d[:].tensor, logit_b_d[:].offset,
                         [[0, 64], [1, VP]])
            nc.sync.dma_start(out=logit_b_s[:], in_=_lb_src)
            ident_s = wpool.tile([128, 128], BF16, tag="ident", name="ident")
            nc.sync.dma_start(out=ident_s[:], in_=ident_d[:])
            ones64 = wpool.tile([1, B], BF16, tag="ones64", name="ones64")
            nc.vector.memset(ones64[:], 1.0)

            with tc.tile_pool(name="w1pool", bufs=1) as w1pool:
                xtT_s = load_chunks(w1pool, xtT_d, T * B, 3, "xtT")
                i2hT_s = load_chunks(w1pool, i2hT_d, NGATE, 3, "i2hT")
                h2hT_s = load_chunks(w1pool, h2hT_d, NGATE, RCN, "h2hT")
                h2attT_s = load_chunks(w1pool, h2attT_d, H, RCN, "h2attT")
                att_lb_s = load_chunks(w1pool, att_lb_d, F, NG, "attlb")
                alpha_s = load_chunks(w1pool, alpha_d, 64, HCN, "alpha")
                a2cT_s = load_chunks(w1pool, a2cT_d, 256, FCN, "a2cT")
                bsel_s = w1pool.tile([B, BMY], BF16, tag="bsel", name="bsel")
                nc.sync.dma_start(out=bsel_s[:], in_=bsel_d[:])
                ctx_b_s = w1pool.tile([1, H], BF16, tag="ctxb", name="ctxb")
                nc.sync.dma_start(out=ctx_b_s[:], in_=ctx_b_d[:])
                onesNBL = w1pool.tile([1, NBL], BF16, tag="onesNBL",
                                      name="onesNBL")
                nc.vector.memset(onesNBL[:], 1.0)
                p_attT = [w1pool.tile([128, NBL], BF16, tag=f"pattT{hc}",
                                      name=f"pattT{hc}")
                          for hc in range(HCN)]
                stat_all = w1pool.tile([128, NG * 8], BF16, tag="stat_all",
                                       name="stat_all")
                nc.vector.memset(stat_all[:], 0.0)
                w_bf = w1pool.tile([BMY, LP], BF16, tag="w_bf", name="w_bf")
                nc.vector.memset(w_bf[:], 0.0)
                c_st = w1pool.tile([B, GC], FP32, tag="c_st", name="c_st")
                statb = w1pool.tile([64, 4], BF16, tag="statb", name="statb")

                def emit_rep(rep):
                    agA_out = agA_out_r[rep]
                    agH_out = agH_out_r[rep]
                    agS_out = agS_out_r[rep]

                    def probe(name, src_ap, shape, dt):
                        if rep == 0 and name in probes:
                            probe_(name, src_ap, shape, dt)

                    nc.vector.memset(c_st[:], 0.0)
                    nc.vector.memset(statb[:], 0.0)
                    hT = hpool.tile([128, RCN * 64], BF16, tag="hT",
                                    name="hT0")
                    nc.vector.memset(hT[:], 0.0)
                    hT_hist = [hT]

                    # ---------- phase 0 ----------
                    with (
                        tc.tile_pool(name=f"ctxpool{rep}", bufs=1) as ctxpool,
                        tc.tile_pool(name=f"stream{rep}", bufs=3) as stream,
                    ):
                        ctxT_s = load_chunks(ctxpool, ctxT_d, H, FCN, "ctxT")
                        QW = 392
                        for q in range(4):
                            n0 = q * QW
                            _pa_tags = ["sums", "mid", "ar", "small"]
                            pa_ps = [psum.tile([128, QW], FP32,
                                               tag=_pa_tags[hc],
                                               name=f"pa{hc}", bufs=1)
                                     for hc in range(HCN)]
                            for fc in range(FCN):
                                at = stream.tile([128, QW], BF16, tag="attTq",
                                                 name="attTq")
                                nc.sync.dma_start(
                                    out=at[:],
                                    in_=attT_d[fc * 128:(fc + 1) * 128,
                                               n0:n0 + QW])
                                for hc in range(HCN):
                                    nc.tensor.matmul(
                                        pa_ps[hc][:],
                                        ctxT_s[fc][:,
                                                   hc * 128:(hc + 1) * 128],
                                        at[:], start=(fc == 0), stop=False)
                            for hc in range(HCN):
                                nc.tensor.matmul(
                                    pa_ps[hc][:],
                                    ctx_b_s[:, hc * 128:(hc + 1) * 128],
                                    onesNBL[:, n0:n0 + QW], start=False,
                                    stop=True)
                                nc.vector.tensor_copy(
                                    p_attT[hc][:, n0:n0 + QW], pa_ps[hc][:])
                    probe("p_attT0", p_attT[0][:], [128, NBL], BF16)

                    # ---------- phase 1 ----------
                    with tc.tile_pool(name=f"work1_{rep}", bufs=1) as work:
                        lg_sbs = {}      # block s -> lg_sb tile
                        nlogZs = {}      # block s -> nlogZ tile

                        def logit_mms(s, c0, c1, gate=None):
                            """Emit logit matmuls for step s, col chunk.
                            `gate` (a tiny SBUF tile DMA'd from the AG
                            staging buffer) delays the matmuls until the
                            collective is in flight, so the scheduler can't
                            hoist them out of the AG stall window."""
                            lg_ps = psum.tile([64, 512], FP32, tag="lg",
                                              name=f"lg_ps_{s}_{c0}", bufs=2)
                            if gate is not None:
                                nc.tensor.matmul(lg_ps[0:1, 0:1],
                                                 gate, gate,
                                                 start=True, stop=True)
                            hTs = hT_hist[s + 1]
                            for rc in range(RCN):
                                nc.tensor.matmul(
                                    lg_ps[:, 0:c1 - c0],
                                    hTs[:, rc * 64:(rc + 1) * 64],
                                    logitT_s[rc][:, c0:c1],
                                    start=(rc == 0), stop=(rc == RCN - 1))
                            lg_sb = lg_sbs[s]
                            nc.vector.scalar_tensor_tensor(
                                lg_sb[:, c0:c1], lg_ps[:, 0:c1 - c0],
                                1.0, logit_b_s[:, c0:c1],
                                op0=ALU.mult, op1=ALU.add)

                        def logit_stats(s):
                            """expsum for block s (|logits| <= ~51, so no
                            max-shift needed before exp); bf16 stat into
                            statb slot (s % 2)."""
                            lg_sb = lg_sbs[s]
                            sl = (s % 2) * 2
                            junk = work.tile([64, VP], BF16, tag="junk",
                                             name="junk", bufs=2)
                            s_f = work.tile([64, 1], FP32, tag="s_f",
                                            name="s_f", bufs=2)
                            nc.scalar.activation(junk[:], lg_sb[:], AF.Exp,
                                                 accum_out=s_f[:])
                            nc.vector.tensor_copy(statb[:, sl + 1:sl + 2],
                                                  s_f[:])

                        def emit_logZ(statg_ap, nj, j0):
                            """Combine gathered per-core expsums -> nlogZ
                            ([64, 1] f32, = -logZ)."""
                            sview = statg_ap.rearrange("p (c j) -> p j c",
                                                       j=nj)
                            S_t = work.tile([64, 1], FP32, tag="S_t",
                                            name="S_t", bufs=2)
                            nc.vector.tensor_reduce(
                                S_t[:], sview[:, j0 + 1:j0 + 2, :],
                                axis=AX.X, op=ALU.add)
                            lnS = work.tile([64, 1], FP32, tag="lnS",
                                            name="lnS", bufs=2)
                            nc.scalar.activation(lnS[:], S_t[:], AF.Ln)
                            nlogZ = work.tile([64, 1], FP32, tag="nlogZ",
                                              name="nlogZ", bufs=2)
                            nc.vector.tensor_scalar(nlogZ[:], lnS[:], -1.0,
                                                    None, op0=ALU.mult)
                            return nlogZ

                        def emit_out(s):
                            """Subtract logZ for block s and DMA out."""
                            lp_t = work.tile([64, VP], FP32, tag="lp_t",
                                             name="lp_t", bufs=2)
                            nc.vector.tensor_scalar(
                                lp_t[:], lg_sbs[s][:], nlogZs[s][:], None,
                                op0=ALU.add)
                            nc.sync.dma_start(
                                out=out_d[s * B:(s + 1) * B, :], in_=lp_t[:])
                            del lg_sbs[s], nlogZs[s]

                        for t in range(t_steps):
                            # ======== segment A: attention ========
                            agAx = work.tile([128, AGW], BF16, tag="agAx",
                                             name="agAx", bufs=2)
                            if t >= 2:
                                nc.vector.tensor_copy(
                                    agAx[0:64, 128:130],
                                    statb[:, ((t - 2) % 2) * 2:
                                          ((t - 2) % 2) * 2 + 2])
                            else:
                                nc.vector.memset(agAx[:, 128:130], 0.0)

                            ah_ps = psum.tile([B, H], FP32, tag="mid",
                                              name="ah_ps", bufs=1)
                            for rc in range(RCN):
                                nc.tensor.matmul(
                                    ah_ps[:], hT[:, rc * 64:(rc + 1) * 64],
                                    h2attT_s[rc][:], start=(rc == 0),
                                    stop=(rc == RCN - 1))
                            ah_sb = work.tile([B, H], BF16, tag="ah_sb",
                                              name="ah_sb", bufs=1)
                            nc.vector.tensor_copy(ah_sb[:], ah_ps[:])
                            ahT_ps = psum.tile([128, HCN * 8], FP32,
                                               tag="small", name="ahT_ps",
                                               bufs=1)
                            for hc in range(HCN):
                                nc.tensor.matmul(
                                    ahT_ps[:, hc * 8:(hc + 1) * 8],
                                    ah_sb[:, hc * 128:(hc + 1) * 128],
                                    bsel_s[:], start=True, stop=True)
                            ahT = work.tile([128, HCN * 8], BF16,
                                            tag="ahT_sb", name="ahT_sb",
                                            bufs=1)
                            nc.vector.tensor_copy(ahT[:], ahT_ps[:])

                            e_ps = psum.tile([BMY, L], FP32, tag="small",
                                             name="e_ps", bufs=1)
                            HB = BMY // 2
                            for hc in range(HCN):
                                dp = work.tile([128, NBL], BF16, tag="dp",
                                               name="dp", bufs=2)
                                dt_ = work.tile([128, NBL], BF16, tag="dt",
                                                name="dt", bufs=2)
                                for bh in range(2):
                                    c0, c1 = bh * HB * L, (bh + 1) * HB * L
                                    nc.vector.tensor_tensor(
                                        dp[:, c0:c1].rearrange(
                                            "p (b l) -> p b l", b=HB),
                                        p_attT[hc][:, c0:c1].rearrange(
                                            "p (b l) -> p b l", b=HB),
                                        bcast_free(
                                            ahT[:, hc * 8 + bh * HB:
                                                hc * 8 + (bh + 1) * HB], L),
                                        op=ALU.add)
                                    nc.scalar.activation(dt_[:, c0:c1],
                                                         dp[:, c0:c1],
                                                         AF.Tanh)
                                    for b in range(bh * HB, (bh + 1) * HB):
                                        nc.tensor.matmul(
                                            e_ps[:],
                                            alpha_s[hc][:,
                                                        b * 8:(b + 1) * 8],
                                            dt_[:, b * L:(b + 1) * L],
                                            start=(hc == 0 and b == 0),
                                            stop=(hc == HCN - 1 and
                                                  b == BMY - 1))

                            # |e| <= ||alpha||_1 ~ 8, so exp(e) is far from
                            # f32/bf16 overflow: skip the max-shift and
                            # write bf16 directly into the scatter source
                            nc.scalar.activation(w_bf[:, 0:L], e_ps[:],
                                                 AF.Exp)

                            # scatter unnormalized weights into the
                            # (lp, b)-diagonal layout via a DRAM round trip
                            # (reads split across the SP and ACT dma
                            # queues); 1/sum folds into the psum copy below
                            wdr = dpool.tile([BMY, LP], BF16, tag="wdr",
                                             name="wdr")
                            nc.sync.dma_start(out=wdr[:], in_=w_bf[:])
                            for b in range(BMY):
                                dmae = nc.sync if b % 2 == 0 else nc.scalar
                                dmae.dma_start(
                                    out=stat_all[b:128:8, b:NG * 8:8],
                                    in_=wdr[b:b + 1, :].rearrange(
                                        "o (g lp) -> (o lp) g", g=NG))

                            ssum = work.tile([BMY, 1], FP32, tag="ssum",
                                             name="ssum", bufs=1)
                            nc.vector.tensor_reduce(ssum[:], w_bf[:, 0:L],
                                                    axis=AX.X, op=ALU.add)
                            rinv = work.tile([BMY, 1], FP32, tag="rinv",
                                             name="rinv", bufs=1)
                            nc.vector.reciprocal(rinv[:], ssum[:])

                            ar_sb = work.tile([BMY, F], BF16, tag="ar_sb",
                                              name="ar_sb", bufs=1)
                            for half in range(2):
                                f0 = half * 1024
                                ar_ps = psum.tile([BMY, 1024], FP32,
                                                  tag="ar", name="ar_ps",
                                                  bufs=1)
                                for g in range(NG):
                                    for qf in range(2):
                                        nc.tensor.matmul(
                                            ar_ps[:,
                                                  qf * 512:(qf + 1) * 512],
                                            stat_all[:, g * 8:(g + 1) * 8],
                                            att_lb_s[g][:,
                                                        f0 + qf * 512:
                                                        f0 + (qf + 1) * 512],
                                            start=(g == 0),
                                            stop=(g == NG - 1))
                                if half == 0:
                                    nc.vector.tensor_scalar(
                                        ar_sb[:, f0:f0 + 1024], ar_ps[:],
                                        rinv[:], None, op0=ALU.mult)
                                else:
                                    nc.scalar.activation(
                                        ar_sb[:, f0:f0 + 1024], ar_ps[:],
                                        AF.Copy, scale=rinv[:])

                            # transpose own att_res before the AllGather;
                            # stage straight from PSUM + statb to DRAM
                            arTo_ps = psum.tile([128, 128], BF16, tag="mid",
                                                name="arTo_ps", bufs=1)
                            for fc in range(FCN):
                                nc.tensor.transpose(
                                    arTo_ps[:, fc * 8:(fc + 1) * 8],
                                    ar_sb[:, fc * 128:(fc + 1) * 128],
                                    ident_s[0:BMY, 0:BMY])
                            nc.vector.tensor_copy(agAx[:, 0:128],
                                                  arTo_ps[:])
                            agA_in = dpool.tile([128, AGW], BF16,
                                                tag="agA_in", name="agA_in")
                            nc.sync.dma_start(out=agA_in[:], in_=agAx[:])
                            if no_cc:
                                nc.sync.dma_start(out=agA_out[t][0:128, :],
                                                  in_=agA_in[:])
                            else:
                                nc.gpsimd.collective_compute(
                                    "AllGather", ALU.bypass,
                                    replica_groups=RG,
                                    ins=[agA_in.opt()], outs=[agA_out[t][:]])

                            # ======== window 1 (during AG_A) ========
                            sums_ps = psum.tile([B, NGATE], FP32, tag="sums",
                                                name="sums", bufs=1)
                            for c0 in (0, 512):
                                c1 = min(NGATE, c0 + 512)
                                for kc in range(3):
                                    nc.tensor.matmul(
                                        sums_ps[:, c0:c1],
                                        xtT_s[kc][:, t * B:(t + 1) * B],
                                        i2hT_s[kc][:, c0:c1],
                                        start=(kc == 0), stop=False)
                                for rc in range(RCN):
                                    nc.tensor.matmul(
                                        sums_ps[:, c0:c1],
                                        hT[:, rc * 64:(rc + 1) * 64],
                                        h2hT_s[rc][:, c0:c1],
                                        start=False, stop=(rc == RCN - 1))
                            sig3 = work.tile([B, 384], FP32, tag="sig3",
                                             name="sig3", bufs=1)
                            nc.scalar.activation(sig3[:], sums_ps[:, 0:384],
                                                 AF.Tanh, scale=0.5)
                            sitr = work.tile([B, 256], FP32, tag="sitr",
                                             name="sitr", bufs=1)
                            nc.vector.tensor_copy(sitr[:],
                                                  sums_ps[:, 384:640])

                            if t >= 1:
                                s = t - 1
                                lg_sbs[s] = work.tile([64, VP], FP32,
                                                      tag="lg_sb",
                                                      name=f"lg_sb{s}",
                                                      bufs=3)
                                gsbA = work.tile([1, 1], BF16, tag="gsbA",
                                                 name="gsbA", bufs=2)
                                nc.sync.dma_start(out=gsbA[:],
                                                  in_=agA_in[0:1, 0:1])
                                logit_mms(s, 0, 512)
                                logit_mms(s, 512, 1024, gate=gsbA[:])

                            # ======== post-AG_A ========
                            # arTc cols: c*128 + fc*8 + b (one DMA),
                            # then DVE repack to fc-major for the matmul
                            arTc = work.tile([128, FCN * 64], BF16,
                                             tag="arTc", name="arTc",
                                             bufs=1)
                            _ag = agA_out[t][:]
                            arT_src = AP(_ag.tensor, _ag.offset,
                                         [[AGW, 128], [128 * AGW, NC],
                                          [1, FCN * BMY]])
                            nc.sync.dma_start(
                                out=arTc[:].rearrange("p (c fb) -> p c fb",
                                                      c=NC),
                                in_=arT_src)
                            arT = work.tile([128, FCN * 64], BF16, tag="arT",
                                            name="arT", bufs=1)
                            nc.vector.tensor_copy(
                                arT[:].rearrange("p (fc c b) -> p fc c b",
                                                 fc=FCN, c=NC),
                                arTc[:].rearrange("p (c fc b) -> p fc c b",
                                                  c=NC, fc=FCN))

                            ctx_ps = psum.tile([B, 256], FP32, tag="mid",
                                               name="ctx_ps", bufs=1)
                            for fc in range(FCN):
                                nc.tensor.matmul(
                                    ctx_ps[:], arT[:, fc * 64:(fc + 1) * 64],
                                    a2cT_s[fc][:], start=(fc == 0),
                                    stop=(fc == FCN - 1))

                            itr1 = work.tile([B, GC], FP32, tag="itr1",
                                             name="itr1", bufs=1)
                            nc.vector.tensor_tensor(itr1[:], sitr[:, 0:128],
                                                    ctx_ps[:, 0:128],
                                                    op=ALU.add)
                            itr2 = work.tile([B, GC], FP32, tag="itr2",
                                             name="itr2", bufs=1)
                            nc.vector.tensor_tensor(itr2[:],
                                                    sitr[:, 128:256],
                                                    ctx_ps[:, 128:256],
                                                    op=ALU.add)
                            g_t = work.tile([B, GC], FP32, tag="g_t",
                                            name="g_t", bufs=1)
                            nc.vector.tensor_tensor(g_t[:], itr1[:],
                                                    itr2[:], op=ALU.max)
                            a_t = work.tile([B, GC], FP32, tag="a_t",
                                            name="a_t", bufs=1)
                            nc.vector.scalar_tensor_tensor(
                                a_t[:], sig3[:, 128:256], 1.0, c_st[:],
                                op0=ALU.add, op1=ALU.mult)
                            b_t = work.tile([B, GC], FP32, tag="b_t",
                                            name="b_t", bufs=1)
                            nc.vector.scalar_tensor_tensor(
                                b_t[:], sig3[:, 0:128], 1.0, g_t[:],
                                op0=ALU.add, op1=ALU.mult)
                            nc2_t = work.tile([B, GC], FP32, tag="nc2",
                                              name="nc2", bufs=1)
                            nc.vector.tensor_tensor(nc2_t[:], a_t[:],
                                                    b_t[:], op=ALU.add)
                            nc.vector.tensor_scalar(c_st[:], nc2_t[:], 0.5,
                                                    None, op0=ALU.mult)
                            tnc = work.tile([B, GC], FP32, tag="tnc",
                                            name="tnc", bufs=1)
                            nc.scalar.activation(tnc[:], nc2_t[:], AF.Tanh,
                                                 scale=0.5)
                            nh2 = work.tile([B, GC], BF16, tag="nh2",
                                            name="nh2", bufs=1)
                            nc.vector.scalar_tensor_tensor(
                                nh2[:], sig3[:, 256:384], 1.0, tnc[:],
                                op0=ALU.add, op1=ALU.mult)

                            nhT_ps = psum.tile([GC, B], BF16, tag="small",
                                               name="nhT_ps", bufs=1)
                            nc.tensor.transpose(nhT_ps[:], nh2[:],
                                                ident_s[0:B, 0:B])
                            nhT_sb = work.tile([GC, B], BF16, tag="nhT_sb",
                                               name="nhT_sb", bufs=1)
                            nc.vector.tensor_copy(nhT_sb[:], nhT_ps[:])
                            agH_in = dpool.tile([GC, B], BF16, tag="agH_in",
                                                name="agH_in")
                            nc.sync.dma_start(out=agH_in[:], in_=nhT_sb[:])
                            if no_cc:
                                nc.sync.dma_start(out=agH_out[t][0:GC, :],
                                                  in_=agH_in[:])
                            else:
                                nc.gpsimd.collective_compute(
                                    "AllGather", ALU.bypass,
                                    replica_groups=RG,
                                    ins=[agH_in.opt()], outs=[agH_out[t][:]])

                            # ======== window 2 (during AG_H) ========
                            gH = agH_in[0:1, 0:1]
                            gsbH = work.tile([1, 1], BF16, tag="gsbH",
                                             name="gsbH", bufs=2)
                            nc.sync.dma_start(out=gsbH[:], in_=gH)
                            if t >= 1:
                                s = t - 1
                                logit_mms(s, 1024, VP, gate=gsbH[:])
                                logit_stats(s)
                            if t >= 2:
                                statg = work.tile([64, 2 * NC], BF16,
                                                  tag="statg", name="statg",
                                                  bufs=2)
                                nc.sync.dma_start(out=statg[0:1, 0:1],
                                                  in_=gH)
                                statg_src = AP(_ag.tensor, _ag.offset + 128,
                                               [[AGW, 64], [128 * AGW, NC],
                                                [1, 2]])
                                nc.sync.dma_start(
                                    out=statg[:].rearrange(
                                        "p (c j) -> p c j", c=NC),
                                    in_=statg_src)
                                nlogZs[t - 2] = emit_logZ(statg[:], 2, 0)
                                emit_out(t - 2)
                                # prefetch the tanh act-table set while the
                                # AG still runs (Ln swapped the set out)
                                dumt = work.tile([1, 1], FP32, tag="dumt",
                                                 name="dumt", bufs=1)
                                nc.scalar.activation(dumt[:],
                                                     statb[0:1, 0:1],
                                                     AF.Tanh)

                            # ======== post-AG_H ========
                            hT_new = hpool.tile([128, RCN * 64], BF16,
                                                tag="hT", name="hT_new")
                            nc.sync.dma_start(
                                out=hT_new[:].rearrange(
                                    "rl (rc b) -> rl rc b", rc=RCN),
                                in_=agH_out[t][:].rearrange(
                                    "(rc rl) b -> rl rc b", rc=RCN))
                            hT_hist.append(hT_new)
                            hT = hT_new

                            if t == 0:
                                probe("ah0", ah_sb[:], [B, H], BF16)
                                probe("w0", w_bf[:], [BMY, LP], BF16)
                                probe("statall0", stat_all[:],
                                      [128, NG * 8], BF16)
                                probe("ar0", ar_sb[:], [BMY, F], BF16)
                                probe("nh20", nh2[:], [B, GC], BF16)
                                probe("hT1", hT_new[:], [128, RCN * 64],
                                      BF16)
                            if t == 1:
                                probe("lg0", lg_sbs[0][:], [64, VP], FP32)
                                probe("arT1", arT[:], [128, FCN * 64], BF16)

                        # ======== tail ========
                        s = t_steps - 1
                        lg_sbs[s] = work.tile([64, VP], FP32, tag="lg_sb",
                                              name=f"lg_sb{s}", bufs=3)
                        logit_mms(s, 0, 512)
                        logit_mms(s, 512, 1024)
                        logit_mms(s, 1024, VP)
                        logit_stats(s)

                        agS_in = dpool.tile([64, 4], BF16, tag="agS_in",
                                            name="agS_in")
                        nc.sync.dma_start(out=agS_in[:], in_=statb[:])
                        if no_cc:
                            nc.sync.dma_start(out=agS_out[0:64, :],
                                              in_=agS_in[:])
                        else:
                            nc.gpsimd.collective_compute(
                                "AllGather", ALU.bypass, replica_groups=RG,
                                ins=[agS_in.opt()], outs=[agS_out[:]])
                        statg2 = work.tile([64, 4 * NC], BF16, tag="statg2",
                                           name="statg2", bufs=1)
                        _ags = agS_out[:]
                        statg2_src = AP(_ags.tensor, _ags.offset,
                                        [[4, 64], [64 * 4, NC], [1, 4]])
                        nc.sync.dma_start(out=statg2[:], in_=statg2_src)
                        for s in (t_steps - 2, t_steps - 1):
                            nlogZs[s] = emit_logZ(statg2[:], 4, (s % 2) * 2)
                            emit_out(s)

                for rep in range(reps):
                    emit_rep(rep)

    nc.compile()
    return nc, sorted(probes)


_NC_CACHE = {}


def kernel(**inputs):
    """Full-input entry point: returns logp [B, T, V1] float32."""
    from concourse.bass_utils import run_bass_kernel_spmd
    in_maps = host_prep(inputs)
    if "nc" not in _NC_CACHE:
        _NC_CACHE["nc"], _ = build(T, (), reps=1)
    nc = _NC_CACHE["nc"]
    res = run_bass_kernel_spmd(nc, in_maps, list(range(NC)))
    outs = [res.results[c]["logp"] for c in range(NC)]
    full = np.concatenate(outs, axis=1)[:, :V1]          # [T*B, V1]
    logp = full.reshape(T, B, V1).transpose(1, 0, 2)
    return np.ascontiguousarray(logp.astype(np.float32))


# revision 73
# speedup vs baseline: 1.2127x; 1.2127x over previous
"""Attention-LSTM captioning model on 8 trn2 cores (8-way tensor parallel).

Gate/itr/vocab output dims sharded across cores (full B=64 per core);
attention batch-sharded (8 batches/core, selected via per-core one-hot bsel
data, since the SPMD program is identical on every core). Activations are
transposed [feature, batch]. Per step: AllGather(att_resT + piggybacked
log-softmax stats), AllGather(nh chunk). Sigmoid(x) is computed as
(tanh(x/2)+1)/2 so the step only needs the {tanh, exp} ACT table; the hidden
state is stored as 2*h with h-consuming weights pre-halved on the host.

Per-step schedule fills both AllGather stall windows: the logit matmul for
step t-1, the gates matmul for step t, per-step log-softmax stats (bf16-
rounded max trick so the stats collapse to 2 bf16 scalars per row riding the
next AG_A), and the final logZ-subtract + output DMA for step t-2.
"""
import numpy as np
import ml_dtypes

import concourse.bacc as bacc
import concourse.mybir as mybir
import concourse.tile as tile
from concourse.ap import AP
from concourse.bass_utils import run_bass_kernel_spmd

BF16_NP = ml_dtypes.bfloat16
FP32 = mybir.dt.float32
BF16 = mybir.dt.bfloat16
AF = mybir.ActivationFunctionType
ALU = mybir.AluOpType
AX = mybir.AxisListType

B, T, R, H, F, E, L, V1 = 64, 20, 1024, 512, 2048, 300, 196, 12001
NC = 8
BMY = B // NC
GC = R // NC              # 128
NGATE = 5 * GC            # 640
VP = 1504
LP = 208
NG = LP // 16             # 13
EP = 384
HCN = H // 128            # 4
FCN = F // 128            # 16
RCN = R // 128            # 8
NBL = BMY * L             # 1568


def _bf(x):
    return np.ascontiguousarray(np.asarray(x, dtype=np.float32)).astype(BF16_NP)


def bcast_free(ap, n):
    """Append a step-0 free dim of size n to an AP (broadcast)."""
    return AP(ap.tensor, ap.offset, list(ap.ap) + [[0, n]])


def host_prep(inputs):
    seq = np.asarray(inputs["seq"])
    att = np.asarray(inputs["att_feats"], dtype=np.float32)
    embed_w = np.asarray(inputs["embed_w"], dtype=np.float32)
    ctx2att_w = np.asarray(inputs["ctx2att_w"], dtype=np.float32)
    ctx2att_b = np.asarray(inputs["ctx2att_b"], dtype=np.float32)
    h2att_w = np.asarray(inputs["h2att_w"], dtype=np.float32)
    h2att_b = np.asarray(inputs["h2att_b"], dtype=np.float32)
    alpha_w = np.asarray(inputs["alpha_w"], dtype=np.float32)
    i2h_w = np.asarray(inputs["i2h_w"], dtype=np.float32)
    i2h_b = np.asarray(inputs["i2h_b"], dtype=np.float32)
    h2h_w = np.asarray(inputs["h2h_w"], dtype=np.float32)
    h2h_b = np.asarray(inputs["h2h_b"], dtype=np.float32)
    a2c_w = np.asarray(inputs["a2c_w"], dtype=np.float32)
    a2c_b = np.asarray(inputs["a2c_b"], dtype=np.float32)
    logit_w = np.asarray(inputs["logit_w"], dtype=np.float32)
    logit_b = np.asarray(inputs["logit_b"], dtype=np.float32)

    xt = embed_w[seq]                                    # [B, T, E]
    xtT = np.zeros((EP, T * B), dtype=np.float32)
    xtT[:E] = xt.transpose(2, 1, 0).reshape(E, T * B)
    xtT[E] = 1.0
    xtT = _bf(xtT)
    bias_gate = i2h_b + h2h_b

    in_maps = []
    for c in range(NC):
        m = {"xtT": xtT}
        grows = np.concatenate([np.arange(gg * R + c * GC, gg * R + (c + 1) * GC)
                                for gg in range(5)])
        i2hT = np.zeros((EP, NGATE), dtype=np.float32)
        i2hT[:E] = i2h_w[grows, :].T
        i2hT[E] = bias_gate[grows]
        # fold the a2c bias into the itr-part gate bias (itr = sums + ctx)
        arows_b = np.concatenate([np.arange(c * GC, (c + 1) * GC),
                                  np.arange(R + c * GC, R + (c + 1) * GC)])
        i2hT[E, 384:640] += a2c_b[arows_b]
        m["i2hT"] = _bf(i2hT)
        m["h2hT"] = _bf(h2h_w[grows, :].T * 0.5)
        m["h2attT"] = _bf(h2att_w.T * 0.5)
        m["ctxT"] = _bf(ctx2att_w.T)
        # h2att_bias folded in: dot = tanh(p_att + h@h2attT) with both
        # biases additive per-h element
        m["ctx_bias"] = _bf((ctx2att_b + h2att_b)[None, :])
        amy = att[c * BMY:(c + 1) * BMY]                 # [8, L, F]
        m["attT"] = _bf(amy.transpose(2, 0, 1).reshape(F, NBL))
        alb = np.zeros((NG * 16, 8, F), dtype=np.float32)
        alb[:L] = amy.transpose(1, 0, 2)
        m["att_lb"] = _bf(alb.reshape(NG * 128, F))
        ac = np.zeros((HCN * 128, 64), dtype=np.float32)
        for b in range(BMY):
            ac[:, b * 8 + b] = alpha_w[0]
        m["alpha_cols"] = _bf(ac)
        arows = np.concatenate([np.arange(c * GC, (c + 1) * GC),
                                np.arange(R + c * GC, R + (c + 1) * GC)])
        m["a2cT"] = _bf(a2c_w[arows, :].T)
        vrows = np.arange(c * VP, (c + 1) * VP)
        lw = np.zeros((R, VP), dtype=np.float32)
        lb = np.full((1, VP), -1e30, dtype=np.float32)
        valid = vrows < V1
        lw[:, valid] = logit_w[vrows[valid], :].T * 0.5
        lb[0, valid] = logit_b[vrows[valid]]
        m["logitT"] = _bf(lw)
        m["logit_bias"] = lb
        m["ident"] = _bf(np.eye(128))
        bsel = np.zeros((B, BMY), dtype=np.float32)
        for j in range(BMY):
            bsel[c * BMY + j, j] = 1.0
        m["bsel"] = _bf(bsel)
        in_maps.append(m)
    return in_maps


def build(t_steps=T, probes=(), reps=1, no_cc=False):
    nc = bacc.Bacc("TRN2", target_bir_lowering=False, debug=False,
                   num_devices=NC)
    probes = set(probes)
    RG = [list(range(NC))]
    AGW = 130                 # agA payload: 128 arT cols + 2 stat cols

    def din(name, shape, dt=BF16):
        return nc.dram_tensor(name, shape, dt, kind="ExternalInput")

    xtT_d = din("xtT", [EP, T * B])
    i2hT_d = din("i2hT", [EP, NGATE])
    h2hT_d = din("h2hT", [R, NGATE])
    h2attT_d = din("h2attT", [R, H])
    ctxT_d = din("ctxT", [F, H])
    ctx_b_d = din("ctx_bias", [1, H])
    attT_d = din("attT", [F, NBL])
    att_lb_d = din("att_lb", [NG * 128, F])
    alpha_d = din("alpha_cols", [HCN * 128, 64])
    a2cT_d = din("a2cT", [F, 256])
    logitT_d = din("logitT", [R, VP])
    logit_b_d = din("logit_bias", [1, VP], FP32)
    ident_d = din("ident", [128, 128])
    bsel_d = din("bsel", [B, BMY])

    out_d = nc.dram_tensor("logp", [t_steps * B, VP], FP32,
                           kind="ExternalOutput")
    agA_out_r = [[nc.dram_tensor(f"agA_out_{rp}_{t}", [NC * 128, AGW], BF16,
                                 addr_space="Shared") for t in range(t_steps)]
                 for rp in range(reps)]
    agH_out_r = [[nc.dram_tensor(f"agH_out_{rp}_{t}", [R, B], BF16,
                                 addr_space="Shared") for t in range(t_steps)]
                 for rp in range(reps)]
    agS_out_r = [nc.dram_tensor(f"agS_out_{rp}", [NC * 64, 4], BF16,
                                addr_space="Shared") for rp in range(reps)]

    with tile.TileContext(nc) as tc:
        with (
            tc.tile_pool(name="wpool", bufs=1) as wpool,
            tc.tile_pool(name="hpool", bufs=4) as hpool,
            tc.tile_pool(name="psum", bufs=1, space="PSUM") as psum,
            tc.tile_pool(name="dram", bufs=4, space="DRAM") as dpool,
        ):
            def probe_(name, src_ap, shape, dt):
                pd = nc.dram_tensor(f"probe_{name}", list(shape), dt,
                                    kind="ExternalOutput")
                nc.sync.dma_start(out=pd[:], in_=src_ap)

            def load_chunks(pool, dram, cols, n, tag, dt=BF16):
                ts = []
                for i in range(n):
                    t_ = pool.tile([128, cols], dt, tag=f"{tag}{i}",
                                   name=f"{tag}{i}")
                    nc.sync.dma_start(out=t_[:],
                                      in_=dram[i * 128:(i + 1) * 128, :])
                    ts.append(t_)
                return ts

            logitT_s = load_chunks(wpool, logitT_d, VP, RCN, "logitT")
            logit_b_s = wpool.tile([64, VP], FP32, tag="logitb",
                                   name="logitb")
            _lb_src = AP(logit_b_d[:].tensor, logit_b_d[:].offset,
                         [[0, 64], [1, VP]])
            nc.sync.dma_start(out=logit_b_s[:], in_=_lb_src)
            ident_s = wpool.tile([128, 128], BF16, tag="ident", name="ident")
            nc.sync.dma_start(out=ident_s[:], in_=ident_d[:])
            ones64 = wpool.tile([1, B], BF16, tag="ones64", name="ones64")
            nc.vector.memset(ones64[:], 1.0)

            with tc.tile_pool(name="w1pool", bufs=1) as w1pool:
                xtT_s = load_chunks(w1pool, xtT_d, T * B, 3, "xtT")
                i2hT_s = load_chunks(w1pool, i2hT_d, NGATE, 3, "i2hT")
                h2hT_s = load_chunks(w1pool, h2hT_d, NGATE, RCN, "h2hT")
                h2attT_s = load_chunks(w1pool, h2attT_d, H, RCN, "h2attT")
                att_lb_s = load_chunks(w1pool, att_lb_d, F, NG, "attlb")
                alpha_s = load_chunks(w1pool, alpha_d, 64, HCN, "alpha")
                a2cT_s = load_chunks(w1pool, a2cT_d, 256, FCN, "a2cT")
                bsel_s = w1pool.tile([B, BMY], BF16, tag="bsel", name="bsel")
                nc.sync.dma_start(out=bsel_s[:], in_=bsel_d[:])
                ctx_b_s = w1pool.tile([1, H], BF16, tag="ctxb", name="ctxb")
                nc.sync.dma_start(out=ctx_b_s[:], in_=ctx_b_d[:])
                onesNBL = w1pool.tile([1, NBL], BF16, tag="onesNBL",
                                      name="onesNBL")
                nc.vector.memset(onesNBL[:], 1.0)
                p_attT = [w1pool.tile([128, NBL], BF16, tag=f"pattT{hc}",
                                      name=f"pattT{hc}")
                          for hc in range(HCN)]
                stat_all = w1pool.tile([128, NG * 8], BF16, tag="stat_all",
                                       name="stat_all")
                nc.vector.memset(stat_all[:], 0.0)
                w_bf = w1pool.tile([BMY, LP], BF16, tag="w_bf", name="w_bf")
                nc.vector.memset(w_bf[:], 0.0)
                c_st = w1pool.tile([B, GC], FP32, tag="c_st", name="c_st")
                statb = w1pool.tile([64, 4], BF16, tag="statb", name="statb")

                def emit_rep(rep):
                    agA_out = agA_out_r[rep]
                    agH_out = agH_out_r[rep]
                    agS_out = agS_out_r[rep]

                    def probe(name, src_ap, shape, dt):
                        if rep == 0 and name in probes:
                            probe_(name, src_ap, shape, dt)

                    nc.vector.memset(c_st[:], 0.0)
                    nc.vector.memset(statb[:], 0.0)
                    hT = hpool.tile([128, RCN * 64], BF16, tag="hT",
                                    name="hT0")
                    nc.vector.memset(hT[:], 0.0)
                    hT_hist = [hT]

                    # ---------- phase 0 ----------
                    with (
                        tc.tile_pool(name=f"ctxpool{rep}", bufs=1) as ctxpool,
                        tc.tile_pool(name=f"stream{rep}", bufs=3) as stream,
                    ):
                        ctxT_s = load_chunks(ctxpool, ctxT_d, H, FCN, "ctxT")
                        QW = 392
                        for q in range(4):
                            n0 = q * QW
                            _pa_tags = ["sums", "mid", "ar", "small"]
                            pa_ps = [psum.tile([128, QW], FP32,
                                               tag=_pa_tags[hc],
                                               name=f"pa{hc}", bufs=1)
                                     for hc in range(HCN)]
                            for fc in range(FCN):
                                at = stream.tile([128, QW], BF16, tag="attTq",
                                                 name="attTq")
                                nc.sync.dma_start(
                                    out=at[:],
                                    in_=attT_d[fc * 128:(fc + 1) * 128,
                                               n0:n0 + QW])
                                for hc in range(HCN):
                                    nc.tensor.matmul(
                                        pa_ps[hc][:],
                                        ctxT_s[fc][:,
                                                   hc * 128:(hc + 1) * 128],
                                        at[:], start=(fc == 0), stop=False)
                            for hc in range(HCN):
                                nc.tensor.matmul(
                                    pa_ps[hc][:],
                                    ctx_b_s[:, hc * 128:(hc + 1) * 128],
                                    onesNBL[:, n0:n0 + QW], start=False,
                                    stop=True)
                                nc.vector.tensor_copy(
                                    p_attT[hc][:, n0:n0 + QW], pa_ps[hc][:])
                    probe("p_attT0", p_attT[0][:], [128, NBL], BF16)

                    # ---------- phase 1 ----------
                    with tc.tile_pool(name=f"work1_{rep}", bufs=1) as work:
                        lg_sbs = {}      # block s -> lg_sb tile
                        nlogZs = {}      # block s -> nlogZ tile

                        def logit_mms(s, c0, c1, gate=None):
                            """Emit logit matmuls for step s, col chunk.
                            `gate` (a tiny SBUF tile DMA'd from the AG
                            staging buffer) delays the matmuls until the
                            collective is in flight, so the scheduler can't
                            hoist them out of the AG stall window."""
                            lg_ps = psum.tile([64, 512], FP32, tag="lg",
                                              name=f"lg_ps_{s}_{c0}", bufs=2)
                            if gate is not None:
                                nc.tensor.matmul(lg_ps[0:1, 0:1],
                                                 gate, gate,
                                                 start=True, stop=True)
                            hTs = hT_hist[s + 1]
                            for rc in range(RCN):
                                nc.tensor.matmul(
                                    lg_ps[:, 0:c1 - c0],
                                    hTs[:, rc * 64:(rc + 1) * 64],
                                    logitT_s[rc][:, c0:c1],
                                    start=(rc == 0), stop=(rc == RCN - 1))
                            lg_sb = lg_sbs[s]
                            nc.vector.scalar_tensor_tensor(
                                lg_sb[:, c0:c1], lg_ps[:, 0:c1 - c0],
                                1.0, logit_b_s[:, c0:c1],
                                op0=ALU.mult, op1=ALU.add)

                        def logit_stats(s):
                            """expsum for block s (|logits| <= ~51, so no
                            max-shift needed before exp); bf16 stat into
                            statb slot (s % 2)."""
                            lg_sb = lg_sbs[s]
                            sl = (s % 2) * 2
                            junk = work.tile([64, VP], BF16, tag="junk",
                                             name="junk", bufs=2)
                            s_f = work.tile([64, 1], FP32, tag="s_f",
                                            name="s_f", bufs=2)
                            nc.scalar.activation(junk[:], lg_sb[:], AF.Exp,
                                                 accum_out=s_f[:])
                            nc.vector.tensor_copy(statb[:, sl + 1:sl + 2],
                                                  s_f[:])

                        def emit_logZ(statg_ap, nj, j0):
                            """Combine gathered per-core expsums -> nlogZ
                            ([64, 1] f32, = -logZ)."""
                            sview = statg_ap.rearrange("p (c j) -> p j c",
                                                       j=nj)
                            S_t = work.tile([64, 1], FP32, tag="S_t",
                                            name="S_t", bufs=2)
                            nc.vector.tensor_reduce(
                                S_t[:], sview[:, j0 + 1:j0 + 2, :],
                                axis=AX.X, op=ALU.add)
                            lnS = work.tile([64, 1], FP32, tag="lnS",
                                            name="lnS", bufs=2)
                            nc.scalar.activation(lnS[:], S_t[:], AF.Ln)
                            nlogZ = work.tile([64, 1], FP32, tag="nlogZ",
                                              name="nlogZ", bufs=2)
                            nc.vector.tensor_scalar(nlogZ[:], lnS[:], -1.0,
                                                    None, op0=ALU.mult)
                            return nlogZ

                        def emit_out(s):
                            """Subtract logZ for block s and DMA out."""
                            lp_t = work.tile([64, VP], FP32, tag="lp_t",
                                             name="lp_t", bufs=2)
                            nc.vector.tensor_scalar(
                                lp_t[:], lg_sbs[s][:], nlogZs[s][:], None,
                                op0=ALU.add)
                            nc.sync.dma_start(
                                out=out_d[s * B:(s + 1) * B, :], in_=lp_t[:])
                            del lg_sbs[s], nlogZs[s]

                        for t in range(t_steps):
                            # ======== segment A: attention ========
                            agAx = work.tile([128, AGW], BF16, tag="agAx",
                                             name="agAx", bufs=2)
                            if t >= 2:
                                nc.vector.tensor_copy(
                                    agAx[0:64, 128:130],
                                    statb[:, ((t - 2) % 2) * 2:
                                          ((t - 2) % 2) * 2 + 2])
                            else:
                                nc.vector.memset(agAx[:, 128:130], 0.0)

                            if t >= 1:
                                ah_ps = psum.tile([B, H], FP32, tag="mid",
                                                  name="ah_ps", bufs=1)
                                for rc in range(RCN):
                                    nc.tensor.matmul(
                                        ah_ps[:],
                                        hT[:, rc * 64:(rc + 1) * 64],
                                        h2attT_s[rc][:], start=(rc == 0),
                                        stop=(rc == RCN - 1))
                                ah_sb = work.tile([B, H], BF16, tag="ah_sb",
                                                  name="ah_sb", bufs=1)
                                nc.vector.tensor_copy(ah_sb[:], ah_ps[:])
                                ahT_ps = psum.tile([128, HCN * 8], FP32,
                                                   tag="small",
                                                   name="ahT_ps", bufs=1)
                                for hc in range(HCN):
                                    nc.tensor.matmul(
                                        ahT_ps[:, hc * 8:(hc + 1) * 8],
                                        ah_sb[:, hc * 128:(hc + 1) * 128],
                                        bsel_s[:], start=True, stop=True)
                                ahT = work.tile([128, HCN * 8], BF16,
                                                tag="ahT_sb", name="ahT_sb",
                                                bufs=1)
                                nc.vector.tensor_copy(ahT[:], ahT_ps[:])

                            e_ps = psum.tile([BMY, L], FP32, tag="small",
                                             name="e_ps", bufs=1)
                            HB = BMY // 2
                            for hc in range(HCN):
                                if t >= 1:
                                    dp = work.tile([128, NBL], BF16,
                                                   tag="dp", name="dp",
                                                   bufs=2)
                                dt_ = work.tile([128, NBL], BF16, tag="dt",
                                                name="dt", bufs=2)
                                for bh in range(2):
                                    c0, c1 = bh * HB * L, (bh + 1) * HB * L
                                    if t == 0:
                                        # h0 == 0 -> dot = tanh(p_att)
                                        nc.scalar.activation(
                                            dt_[:, c0:c1],
                                            p_attT[hc][:, c0:c1], AF.Tanh)
                                    else:
                                        nc.vector.tensor_tensor(
                                            dp[:, c0:c1].rearrange(
                                                "p (b l) -> p b l", b=HB),
                                            p_attT[hc][:, c0:c1].rearrange(
                                                "p (b l) -> p b l", b=HB),
                                            bcast_free(
                                                ahT[:, hc * 8 + bh * HB:
                                                    hc * 8 + (bh + 1) * HB],
                                                L),
                                            op=ALU.add)
                                        nc.scalar.activation(dt_[:, c0:c1],
                                                             dp[:, c0:c1],
                                                             AF.Tanh)
                                    for b in range(bh * HB, (bh + 1) * HB):
                                        nc.tensor.matmul(
                                            e_ps[:],
                                            alpha_s[hc][:,
                                                        b * 8:(b + 1) * 8],
                                            dt_[:, b * L:(b + 1) * L],
                                            start=(hc == 0 and b == 0),
                                            stop=(hc == HCN - 1 and
                                                  b == BMY - 1))

                            # |e| <= ||alpha||_1 ~ 8, so exp(e) is far from
                            # f32/bf16 overflow: skip the max-shift and
                            # write bf16 directly into the scatter source
                            nc.scalar.activation(w_bf[:, 0:L], e_ps[:],
                                                 AF.Exp)

                            # scatter unnormalized weights into the
                            # (lp, b)-diagonal layout via a DRAM round trip
                            # (reads split across the SP and ACT dma
                            # queues); 1/sum folds into the psum copy below
                            wdr = dpool.tile([BMY, LP], BF16, tag="wdr",
                                             name="wdr")
                            nc.sync.dma_start(out=wdr[:], in_=w_bf[:])
                            for b in range(BMY):
                                dmae = nc.sync if b % 2 == 0 else nc.scalar
                                dmae.dma_start(
                                    out=stat_all[b:128:8, b:NG * 8:8],
                                    in_=wdr[b:b + 1, :].rearrange(
                                        "o (g lp) -> (o lp) g", g=NG))

                            ssum = work.tile([BMY, 1], FP32, tag="ssum",
                                             name="ssum", bufs=1)
                            nc.vector.tensor_reduce(ssum[:], w_bf[:, 0:L],
                                                    axis=AX.X, op=ALU.add)
                            rinv = work.tile([BMY, 1], FP32, tag="rinv",
                                             name="rinv", bufs=1)
                            nc.vector.reciprocal(rinv[:], ssum[:])

                            ar_sb = work.tile([BMY, F], BF16, tag="ar_sb",
                                              name="ar_sb", bufs=1)
                            for half in range(2):
                                f0 = half * 1024
                                ar_ps = psum.tile([BMY, 1024], FP32,
                                                  tag="ar", name="ar_ps",
                                                  bufs=1)
                                for g in range(NG):
                                    for qf in range(2):
                                        nc.tensor.matmul(
                                            ar_ps[:,
                                                  qf * 512:(qf + 1) * 512],
                                            stat_all[:, g * 8:(g + 1) * 8],
                                            att_lb_s[g][:,
                                                        f0 + qf * 512:
                                                        f0 + (qf + 1) * 512],
                                            start=(g == 0),
                                            stop=(g == NG - 1))
                                if half == 0:
                                    nc.vector.tensor_scalar(
                                        ar_sb[:, f0:f0 + 1024], ar_ps[:],
                                        rinv[:], None, op0=ALU.mult)
                                else:
                                    nc.scalar.activation(
                                        ar_sb[:, f0:f0 + 1024], ar_ps[:],
                                        AF.Copy, scale=rinv[:])

                            # transpose own att_res before the AllGather;
                            # stage straight from PSUM + statb to DRAM
                            arTo_ps = psum.tile([128, 128], BF16, tag="mid",
                                                name="arTo_ps", bufs=1)
                            for fc in range(FCN):
                                nc.tensor.transpose(
                                    arTo_ps[:, fc * 8:(fc + 1) * 8],
                                    ar_sb[:, fc * 128:(fc + 1) * 128],
                                    ident_s[0:BMY, 0:BMY])
                            nc.vector.tensor_copy(agAx[:, 0:128],
                                                  arTo_ps[:])
                            agA_in = dpool.tile([128, AGW], BF16,
                                                tag="agA_in", name="agA_in")
                            nc.sync.dma_start(out=agA_in[:], in_=agAx[:])
                            if no_cc:
                                nc.sync.dma_start(out=agA_out[t][0:128, :],
                                                  in_=agA_in[:])
                            else:
                                nc.gpsimd.collective_compute(
                                    "AllGather", ALU.bypass,
                                    replica_groups=RG,
                                    ins=[agA_in.opt()], outs=[agA_out[t][:]])

                            # ======== window 1 (during AG_A) ========
                            sums_ps = psum.tile([B, NGATE], FP32, tag="sums",
                                                name="sums", bufs=1)
                            for c0 in (0, 512):
                                c1 = min(NGATE, c0 + 512)
                                for kc in range(3):
                                    nc.tensor.matmul(
                                        sums_ps[:, c0:c1],
                                        xtT_s[kc][:, t * B:(t + 1) * B],
                                        i2hT_s[kc][:, c0:c1],
                                        start=(kc == 0),
                                        stop=(t == 0 and kc == 2))
                                if t >= 1:
                                    for rc in range(RCN):
                                        nc.tensor.matmul(
                                            sums_ps[:, c0:c1],
                                            hT[:, rc * 64:(rc + 1) * 64],
                                            h2hT_s[rc][:, c0:c1],
                                            start=False,
                                            stop=(rc == RCN - 1))
                            sig3 = work.tile([B, 384], FP32, tag="sig3",
                                             name="sig3", bufs=1)
                            nc.scalar.activation(sig3[:], sums_ps[:, 0:384],
                                                 AF.Tanh, scale=0.5)
                            sitr = work.tile([B, 256], FP32, tag="sitr",
                                             name="sitr", bufs=1)
                            nc.vector.tensor_copy(sitr[:],
                                                  sums_ps[:, 384:640])

                            if t >= 1:
                                s = t - 1
                                lg_sbs[s] = work.tile([64, VP], FP32,
                                                      tag="lg_sb",
                                                      name=f"lg_sb{s}",
                                                      bufs=3)
                                gsbA = work.tile([1, 1], BF16, tag="gsbA",
                                                 name="gsbA", bufs=2)
                                nc.sync.dma_start(out=gsbA[:],
                                                  in_=agA_in[0:1, 0:1])
                                logit_mms(s, 0, 512)
                                logit_mms(s, 512, 1024, gate=gsbA[:])

                            # ======== post-AG_A ========
                            # arTc cols: c*128 + fc*8 + b (one DMA),
                            # then DVE repack to fc-major for the matmul
                            arTc = work.tile([128, FCN * 64], BF16,
                                             tag="arTc", name="arTc",
                                             bufs=1)
                            _ag = agA_out[t][:]
                            arT_src = AP(_ag.tensor, _ag.offset,
                                         [[AGW, 128], [128 * AGW, NC],
                                          [1, FCN * BMY]])
                            nc.sync.dma_start(
                                out=arTc[:].rearrange("p (c fb) -> p c fb",
                                                      c=NC),
                                in_=arT_src)
                            arT = work.tile([128, FCN * 64], BF16, tag="arT",
                                            name="arT", bufs=1)
                            nc.vector.tensor_copy(
                                arT[:].rearrange("p (fc c b) -> p fc c b",
                                                 fc=FCN, c=NC),
                                arTc[:].rearrange("p (c fc b) -> p fc c b",
                                                  c=NC, fc=FCN))

                            ctx_ps = psum.tile([B, 256], FP32, tag="mid",
                                               name="ctx_ps", bufs=1)
                            for fc in range(FCN):
                                nc.tensor.matmul(
                                    ctx_ps[:], arT[:, fc * 64:(fc + 1) * 64],
                                    a2cT_s[fc][:], start=(fc == 0),
                                    stop=(fc == FCN - 1))

                            itr1 = work.tile([B, GC], FP32, tag="itr1",
                                             name="itr1", bufs=1)
                            nc.vector.tensor_tensor(itr1[:], sitr[:, 0:128],
                                                    ctx_ps[:, 0:128],
                                                    op=ALU.add)
                            itr2 = work.tile([B, GC], FP32, tag="itr2",
                                             name="itr2", bufs=1)
                            nc.vector.tensor_tensor(itr2[:],
                                                    sitr[:, 128:256],
                                                    ctx_ps[:, 128:256],
                                                    op=ALU.add)
                            g_t = work.tile([B, GC], FP32, tag="g_t",
                                            name="g_t", bufs=1)
                            nc.vector.tensor_tensor(g_t[:], itr1[:],
                                                    itr2[:], op=ALU.max)
                            a_t = work.tile([B, GC], FP32, tag="a_t",
                                            name="a_t", bufs=1)
                            nc.vector.scalar_tensor_tensor(
                                a_t[:], sig3[:, 128:256], 1.0, c_st[:],
                                op0=ALU.add, op1=ALU.mult)
                            b_t = work.tile([B, GC], FP32, tag="b_t",
                                            name="b_t", bufs=1)
                            nc.vector.scalar_tensor_tensor(
                                b_t[:], sig3[:, 0:128], 1.0, g_t[:],
                                op0=ALU.add, op1=ALU.mult)
                            nc2_t = work.tile([B, GC], FP32, tag="nc2",
                                              name="nc2", bufs=1)
                            nc.vector.tensor_tensor(nc2_t[:], a_t[:],
                                                    b_t[:], op=ALU.add)
                            nc.vector.tensor_scalar(c_st[:], nc2_t[:], 0.5,
                                                    None, op0=ALU.mult)
                            tnc = work.tile([B, GC], FP32, tag="tnc",
                                            name="tnc", bufs=1)
                            nc.scalar.activation(tnc[:], nc2_t[:], AF.Tanh,
                                                 scale=0.5)
                            nh2 = work.tile([B, GC], BF16, tag="nh2",
                                            name="nh2", bufs=1)
                            nc.vector.scalar_tensor_tensor(
                                nh2[:], sig3[:, 256:384], 1.0, tnc[:],
                                op0=ALU.add, op1=ALU.mult)

                            nhT_ps = psum.tile([GC, B], BF16, tag="small",
                                               name="nhT_ps", bufs=1)
                            nc.tensor.transpose(nhT_ps[:], nh2[:],
                                                ident_s[0:B, 0:B])
                            nhT_sb = work.tile([GC, B], BF16, tag="nhT_sb",
                                               name="nhT_sb", bufs=1)
                            nc.vector.tensor_copy(nhT_sb[:], nhT_ps[:])
                            agH_in = dpool.tile([GC, B], BF16, tag="agH_in",
                                                name="agH_in")
                            nc.sync.dma_start(out=agH_in[:], in_=nhT_sb[:])
                            if no_cc:
                                nc.sync.dma_start(out=agH_out[t][0:GC, :],
                                                  in_=agH_in[:])
                            else:
                                nc.gpsimd.collective_compute(
                                    "AllGather", ALU.bypass,
                                    replica_groups=RG,
                                    ins=[agH_in.opt()], outs=[agH_out[t][:]])

                            # ======== window 2 (during AG_H) ========
                            gH = agH_in[0:1, 0:1]
                            gsbH = work.tile([1, 1], BF16, tag="gsbH",
                                             name="gsbH", bufs=2)
                            nc.sync.dma_start(out=gsbH[:], in_=gH)
                            if t >= 1:
                                s = t - 1
                                logit_mms(s, 1024, VP, gate=gsbH[:])
                                logit_stats(s)
                            if t >= 2:
                                statg = work.tile([64, 2 * NC], BF16,
                                                  tag="statg", name="statg",
                                                  bufs=2)
                                nc.sync.dma_start(out=statg[0:1, 0:1],
                                                  in_=gH)
                                statg_src = AP(_ag.tensor, _ag.offset + 128,
                                               [[AGW, 64], [128 * AGW, NC],
                                                [1, 2]])
                                nc.sync.dma_start(
                                    out=statg[:].rearrange(
                                        "p (c j) -> p c j", c=NC),
                                    in_=statg_src)
                                nlogZs[t - 2] = emit_logZ(statg[:], 2, 0)
                                emit_out(t - 2)
                                # prefetch the tanh act-table set while the
                                # AG still runs (Ln swapped the set out)
                                dumt = work.tile([1, 1], FP32, tag="dumt",
                                                 name="dumt", bufs=1)
                                nc.scalar.activation(dumt[:],
                                                     statb[0:1, 0:1],
                                                     AF.Tanh)

                            # ======== post-AG_H ========
                            hT_new = hpool.tile([128, RCN * 64], BF16,
                                                tag="hT", name="hT_new")
                            nc.sync.dma_start(
                                out=hT_new[:].rearrange(
                                    "rl (rc b) -> rl rc b", rc=RCN),
                                in_=agH_out[t][:].rearrange(
                                    "(rc rl) b -> rl rc b", rc=RCN))
                            hT_hist.append(hT_new)
                            hT = hT_new

                            if t == 0:
                                probe("w0", w_bf[:], [BMY, LP], BF16)
                                probe("statall0", stat_all[:],
                                      [128, NG * 8], BF16)
                                probe("ar0", ar_sb[:], [BMY, F], BF16)
                                probe("nh20", nh2[:], [B, GC], BF16)
                                probe("hT1", hT_new[:], [128, RCN * 64],
                                      BF16)
                            if t == 1:
                                probe("lg0", lg_sbs[0][:], [64, VP], FP32)
                                probe("arT1", arT[:], [128, FCN * 64], BF16)

                        # ======== tail ========
                        s = t_steps - 1
                        lg_sbs[s] = work.tile([64, VP], FP32, tag="lg_sb",
                                              name=f"lg_sb{s}", bufs=3)
                        logit_mms(s, 0, 512)
                        logit_mms(s, 512, 1024)
                        logit_mms(s, 1024, VP)
                        logit_stats(s)

                        agS_in = dpool.tile([64, 4], BF16, tag="agS_in",
                                            name="agS_in")
                        nc.sync.dma_start(out=agS_in[:], in_=statb[:])
                        if no_cc:
                            nc.sync.dma_start(out=agS_out[0:64, :],
                                              in_=agS_in[:])
                        else:
                            nc.gpsimd.collective_compute(
                                "AllGather", ALU.bypass, replica_groups=RG,
                                ins=[agS_in.opt()], outs=[agS_out[:]])
                        statg2 = work.tile([64, 4 * NC], BF16, tag="statg2",
                                           name="statg2", bufs=1)
                        _ags = agS_out[:]
                        statg2_src = AP(_ags.tensor, _ags.offset,
                                        [[4, 64], [64 * 4, NC], [1, 4]])
                        nc.sync.dma_start(out=statg2[:], in_=statg2_src)
                        for s in (t_steps - 2, t_steps - 1):
                            nlogZs[s] = emit_logZ(statg2[:], 4, (s % 2) * 2)
                            emit_out(s)

                for rep in range(reps):
                    emit_rep(rep)

    nc.compile()
    return nc, sorted(probes)


_NC_CACHE = {}


def kernel(**inputs):
    """Full-input entry point: returns logp [B, T, V1] float32."""
    from concourse.bass_utils import run_bass_kernel_spmd
    in_maps = host_prep(inputs)
    if "nc" not in _NC_CACHE:
        _NC_CACHE["nc"], _ = build(T, (), reps=1)
    nc = _NC_CACHE["nc"]
    res = run_bass_kernel_spmd(nc, in_maps, list(range(NC)))
    outs = [res.results[c]["logp"] for c in range(NC)]
    full = np.concatenate(outs, axis=1)[:, :V1]          # [T*B, V1]
    logp = full.reshape(T, B, V1).transpose(1, 0, 2)
    return np.ascontiguousarray(logp.astype(np.float32))


# revision 79
# speedup vs baseline: 1.5537x; 1.2812x over previous
"""Attention-LSTM captioning model on 8 trn2 cores (8-way tensor parallel).

Gate/itr/vocab output dims sharded across cores (full B=64 per core);
attention batch-sharded (8 batches/core, selected via per-core one-hot bsel
data, since the SPMD program is identical on every core). Activations are
transposed [feature, batch]. Per step: AllGather(att_resT + piggybacked
log-softmax stats), AllGather(nh chunk). Sigmoid(x) is computed as
(tanh(x/2)+1)/2 so the step only needs the {tanh, exp} ACT table; the hidden
state is stored as 2*h with h-consuming weights pre-halved on the host.

Per-step schedule fills both AllGather stall windows: the logit matmul for
step t-1, the gates matmul for step t, per-step log-softmax stats (bf16-
rounded max trick so the stats collapse to 2 bf16 scalars per row riding the
next AG_A), and the final logZ-subtract + output DMA for step t-2.
"""
import numpy as np
import ml_dtypes

import concourse.bacc as bacc
import concourse.mybir as mybir
import concourse.tile as tile
from concourse.ap import AP
from concourse.bass_utils import run_bass_kernel_spmd

BF16_NP = ml_dtypes.bfloat16
FP32 = mybir.dt.float32
BF16 = mybir.dt.bfloat16
AF = mybir.ActivationFunctionType
ALU = mybir.AluOpType
AX = mybir.AxisListType

B, T, R, H, F, E, L, V1 = 64, 20, 1024, 512, 2048, 300, 196, 12001
NC = 8
BMY = B // NC
GC = R // NC              # 128
NGATE = 5 * GC            # 640
VP = 1504
LP = 208
NG = LP // 16             # 13
EP = 384
HCN = H // 128            # 4
FCN = F // 128            # 16
RCN = R // 128            # 8
NBL = BMY * L             # 1568


def _bf(x):
    return np.ascontiguousarray(np.asarray(x, dtype=np.float32)).astype(BF16_NP)


def bcast_free(ap, n):
    """Append a step-0 free dim of size n to an AP (broadcast)."""
    return AP(ap.tensor, ap.offset, list(ap.ap) + [[0, n]])


def host_prep(inputs):
    seq = np.asarray(inputs["seq"])
    att = np.asarray(inputs["att_feats"], dtype=np.float32)
    embed_w = np.asarray(inputs["embed_w"], dtype=np.float32)
    ctx2att_w = np.asarray(inputs["ctx2att_w"], dtype=np.float32)
    ctx2att_b = np.asarray(inputs["ctx2att_b"], dtype=np.float32)
    h2att_w = np.asarray(inputs["h2att_w"], dtype=np.float32)
    h2att_b = np.asarray(inputs["h2att_b"], dtype=np.float32)
    alpha_w = np.asarray(inputs["alpha_w"], dtype=np.float32)
    i2h_w = np.asarray(inputs["i2h_w"], dtype=np.float32)
    i2h_b = np.asarray(inputs["i2h_b"], dtype=np.float32)
    h2h_w = np.asarray(inputs["h2h_w"], dtype=np.float32)
    h2h_b = np.asarray(inputs["h2h_b"], dtype=np.float32)
    a2c_w = np.asarray(inputs["a2c_w"], dtype=np.float32)
    a2c_b = np.asarray(inputs["a2c_b"], dtype=np.float32)
    logit_w = np.asarray(inputs["logit_w"], dtype=np.float32)
    logit_b = np.asarray(inputs["logit_b"], dtype=np.float32)

    xt = embed_w[seq]                                    # [B, T, E]
    xtT = np.zeros((EP, T * B), dtype=np.float32)
    xtT[:E] = xt.transpose(2, 1, 0).reshape(E, T * B)
    xtT[E] = 1.0
    xtT = _bf(xtT)
    bias_gate = i2h_b + h2h_b

    in_maps = []
    for c in range(NC):
        m = {"xtT": xtT}
        grows = np.concatenate([np.arange(gg * R + c * GC, gg * R + (c + 1) * GC)
                                for gg in range(5)])
        i2hT = np.zeros((EP, NGATE), dtype=np.float32)
        i2hT[:E] = i2h_w[grows, :].T
        i2hT[E] = bias_gate[grows]
        # fold the a2c bias into the itr-part gate bias (itr = sums + ctx)
        arows_b = np.concatenate([np.arange(c * GC, (c + 1) * GC),
                                  np.arange(R + c * GC, R + (c + 1) * GC)])
        i2hT[E, 384:640] += a2c_b[arows_b]
        m["i2hT"] = _bf(i2hT)
        m["h2hT"] = _bf(h2h_w[grows, :].T * 0.5)
        m["h2attT"] = _bf(h2att_w.T * 0.5)
        m["ctxT"] = _bf(ctx2att_w.T)
        # h2att_bias folded in: dot = tanh(p_att + h@h2attT) with both
        # biases additive per-h element
        m["ctx_bias"] = _bf((ctx2att_b + h2att_b)[None, :])
        amy = att[c * BMY:(c + 1) * BMY]                 # [8, L, F]
        m["attT"] = _bf(amy.transpose(2, 0, 1).reshape(F, NBL))
        alb = np.zeros((NG * 16, 8, F), dtype=np.float32)
        alb[:L] = amy.transpose(1, 0, 2)
        m["att_lb"] = _bf(alb.reshape(NG * 128, F))
        ac = np.zeros((HCN * 128, 64), dtype=np.float32)
        for b in range(BMY):
            ac[:, b * 8 + b] = alpha_w[0]
        m["alpha_cols"] = _bf(ac)
        arows = np.concatenate([np.arange(c * GC, (c + 1) * GC),
                                np.arange(R + c * GC, R + (c + 1) * GC)])
        m["a2cT"] = _bf(a2c_w[arows, :].T)
        vrows = np.arange(c * VP, (c + 1) * VP)
        lw = np.zeros((R, VP), dtype=np.float32)
        lb = np.full((1, VP), -1e30, dtype=np.float32)
        valid = vrows < V1
        lw[:, valid] = logit_w[vrows[valid], :].T * 0.5
        lb[0, valid] = logit_b[vrows[valid]]
        m["logitT"] = _bf(lw)
        m["logit_bias"] = lb
        m["ident"] = _bf(np.eye(128))
        bsel = np.zeros((B, BMY), dtype=np.float32)
        for j in range(BMY):
            bsel[c * BMY + j, j] = 1.0
        m["bsel"] = _bf(bsel)
        in_maps.append(m)
    return in_maps


def build(t_steps=T, probes=(), reps=1, no_cc=False):
    nc = bacc.Bacc("TRN2", target_bir_lowering=False, debug=False,
                   num_devices=NC)
    probes = set(probes)
    RG = [list(range(NC))]
    AGW = 130                 # agA payload: 128 arT cols + 2 stat cols

    def din(name, shape, dt=BF16):
        return nc.dram_tensor(name, shape, dt, kind="ExternalInput")

    xtT_d = din("xtT", [EP, T * B])
    i2hT_d = din("i2hT", [EP, NGATE])
    h2hT_d = din("h2hT", [R, NGATE])
    h2attT_d = din("h2attT", [R, H])
    ctxT_d = din("ctxT", [F, H])
    ctx_b_d = din("ctx_bias", [1, H])
    attT_d = din("attT", [F, NBL])
    att_lb_d = din("att_lb", [NG * 128, F])
    alpha_d = din("alpha_cols", [HCN * 128, 64])
    a2cT_d = din("a2cT", [F, 256])
    logitT_d = din("logitT", [R, VP])
    logit_b_d = din("logit_bias", [1, VP], FP32)
    ident_d = din("ident", [128, 128])
    bsel_d = din("bsel", [B, BMY])

    out_d = nc.dram_tensor("logp", [t_steps * B, VP], FP32,
                           kind="ExternalOutput")
    agA_out_r = [[nc.dram_tensor(f"agA_out_{rp}_{t}", [NC * 128, AGW], BF16,
                                 addr_space="Shared") for t in range(t_steps)]
                 for rp in range(reps)]
    agH_out_r = [[nc.dram_tensor(f"agH_out_{rp}_{t}", [R, B], BF16,
                                 addr_space="Shared") for t in range(t_steps)]
                 for rp in range(reps)]
    agS_out_r = [nc.dram_tensor(f"agS_out_{rp}", [NC * 64, 4], BF16,
                                addr_space="Shared") for rp in range(reps)]

    with tile.TileContext(nc) as tc:
        with (
            tc.tile_pool(name="wpool", bufs=1) as wpool,
            tc.tile_pool(name="hpool", bufs=4) as hpool,
            tc.tile_pool(name="psum", bufs=1, space="PSUM") as psum,
            tc.tile_pool(name="dram", bufs=4, space="DRAM") as dpool,
        ):
            def probe_(name, src_ap, shape, dt):
                pd = nc.dram_tensor(f"probe_{name}", list(shape), dt,
                                    kind="ExternalOutput")
                nc.sync.dma_start(out=pd[:], in_=src_ap)

            def load_chunks(pool, dram, cols, n, tag, dt=BF16):
                ts = []
                for i in range(n):
                    t_ = pool.tile([128, cols], dt, tag=f"{tag}{i}",
                                   name=f"{tag}{i}")
                    nc.sync.dma_start(out=t_[:],
                                      in_=dram[i * 128:(i + 1) * 128, :])
                    ts.append(t_)
                return ts

            logitT_s = load_chunks(wpool, logitT_d, VP, RCN, "logitT")
            logit_b_s = wpool.tile([64, VP], FP32, tag="logitb",
                                   name="logitb")
            _lb_src = AP(logit_b_d[:].tensor, logit_b_d[:].offset,
                         [[0, 64], [1, VP]])
            nc.sync.dma_start(out=logit_b_s[:], in_=_lb_src)
            ident_s = wpool.tile([128, 128], BF16, tag="ident", name="ident")
            nc.sync.dma_start(out=ident_s[:], in_=ident_d[:])
            ones64 = wpool.tile([1, B], BF16, tag="ones64", name="ones64")
            nc.vector.memset(ones64[:], 1.0)

            with tc.tile_pool(name="w1pool", bufs=1) as w1pool:
                xtT_s = load_chunks(w1pool, xtT_d, T * B, 3, "xtT")
                i2hT_s = load_chunks(w1pool, i2hT_d, NGATE, 3, "i2hT")
                h2hT_s = load_chunks(w1pool, h2hT_d, NGATE, RCN, "h2hT")
                h2attT_s = load_chunks(w1pool, h2attT_d, H, RCN, "h2attT")
                att_lb_s = load_chunks(w1pool, att_lb_d, F, NG, "attlb")
                alpha_s = load_chunks(w1pool, alpha_d, 64, HCN, "alpha")
                a2cT_s = load_chunks(w1pool, a2cT_d, 256, FCN, "a2cT")
                bsel_s = w1pool.tile([B, BMY], BF16, tag="bsel", name="bsel")
                nc.sync.dma_start(out=bsel_s[:], in_=bsel_d[:])
                ctx_b_s = w1pool.tile([1, H], BF16, tag="ctxb", name="ctxb")
                nc.sync.dma_start(out=ctx_b_s[:], in_=ctx_b_d[:])
                onesNBL = w1pool.tile([1, NBL], BF16, tag="onesNBL",
                                      name="onesNBL")
                nc.vector.memset(onesNBL[:], 1.0)
                p_attT = [w1pool.tile([128, NBL], BF16, tag=f"pattT{hc}",
                                      name=f"pattT{hc}")
                          for hc in range(HCN)]
                stat_all = w1pool.tile([128, NG * 8], BF16, tag="stat_all",
                                       name="stat_all")
                nc.vector.memset(stat_all[:], 0.0)
                w_bf = w1pool.tile([BMY, LP], BF16, tag="w_bf", name="w_bf")
                nc.vector.memset(w_bf[:], 0.0)
                c_st = w1pool.tile([B, GC], FP32, tag="c_st", name="c_st")
                statb = w1pool.tile([64, 4], BF16, tag="statb", name="statb")

                def emit_rep(rep):
                    agA_out = agA_out_r[rep]
                    agH_out = agH_out_r[rep]
                    agS_out = agS_out_r[rep]

                    def probe(name, src_ap, shape, dt):
                        if rep == 0 and name in probes:
                            probe_(name, src_ap, shape, dt)

                    nc.vector.memset(c_st[:], 0.0)
                    nc.vector.memset(statb[:], 0.0)
                    hT = hpool.tile([128, RCN * 64], BF16, tag="hT",
                                    name="hT0")
                    nc.vector.memset(hT[:], 0.0)
                    hT_hist = [hT]

                    # ---------- phase 0 ----------
                    with (
                        tc.tile_pool(name=f"ctxpool{rep}", bufs=1) as ctxpool,
                        tc.tile_pool(name=f"stream{rep}", bufs=3) as stream,
                    ):
                        ctxT_s = load_chunks(ctxpool, ctxT_d, H, FCN, "ctxT")
                        QW = 392
                        for q in range(4):
                            n0 = q * QW
                            _pa_tags = ["sums", "mid", "ar", "small"]
                            pa_ps = [psum.tile([128, QW], FP32,
                                               tag=_pa_tags[hc],
                                               name=f"pa{hc}", bufs=1)
                                     for hc in range(HCN)]
                            for fc in range(FCN):
                                at = stream.tile([128, QW], BF16, tag="attTq",
                                                 name="attTq")
                                nc.sync.dma_start(
                                    out=at[:],
                                    in_=attT_d[fc * 128:(fc + 1) * 128,
                                               n0:n0 + QW])
                                for hc in range(HCN):
                                    nc.tensor.matmul(
                                        pa_ps[hc][:],
                                        ctxT_s[fc][:,
                                                   hc * 128:(hc + 1) * 128],
                                        at[:], start=(fc == 0), stop=False)
                            for hc in range(HCN):
                                nc.tensor.matmul(
                                    pa_ps[hc][:],
                                    ctx_b_s[:, hc * 128:(hc + 1) * 128],
                                    onesNBL[:, n0:n0 + QW], start=False,
                                    stop=True)
                                nc.vector.tensor_copy(
                                    p_attT[hc][:, n0:n0 + QW], pa_ps[hc][:])
                    probe("p_attT0", p_attT[0][:], [128, NBL], BF16)

                    # ---------- phase 1 ----------
                    with tc.tile_pool(name=f"work1_{rep}", bufs=1) as work:
                        lg_sbs = {}      # block s -> lg_sb tile
                        nlogZs = {}      # block s -> nlogZ tile

                        def logit_mms(s, c0, c1, gate=None):
                            """Emit logit matmuls for step s, col chunk.
                            `gate` (a tiny SBUF tile DMA'd from the AG
                            staging buffer) delays the matmuls until the
                            collective is in flight, so the scheduler can't
                            hoist them out of the AG stall window."""
                            lg_ps = psum.tile([64, 512], FP32, tag="lg",
                                              name=f"lg_ps_{s}_{c0}", bufs=2)
                            if gate is not None:
                                nc.tensor.matmul(lg_ps[0:1, 0:1],
                                                 gate, gate,
                                                 start=True, stop=True)
                            hTs = hT_hist[s + 1]
                            for rc in range(RCN):
                                nc.tensor.matmul(
                                    lg_ps[:, 0:c1 - c0],
                                    hTs[:, rc * 64:(rc + 1) * 64],
                                    logitT_s[rc][:, c0:c1],
                                    start=(rc == 0), stop=(rc == RCN - 1))
                            lg_sb = lg_sbs[s]
                            nc.vector.scalar_tensor_tensor(
                                lg_sb[:, c0:c1], lg_ps[:, 0:c1 - c0],
                                1.0, logit_b_s[:, c0:c1],
                                op0=ALU.mult, op1=ALU.add)

                        def logit_stats(s):
                            """expsum for block s (|logits| <= ~51, so no
                            max-shift needed before exp); bf16 stat into
                            statb slot (s % 2)."""
                            lg_sb = lg_sbs[s]
                            sl = (s % 2) * 2
                            junk = work.tile([64, VP], BF16, tag="junk",
                                             name="junk", bufs=2)
                            s_f = work.tile([64, 1], FP32, tag="s_f",
                                            name="s_f", bufs=2)
                            nc.scalar.activation(junk[:], lg_sb[:], AF.Exp,
                                                 accum_out=s_f[:])
                            nc.vector.tensor_copy(statb[:, sl + 1:sl + 2],
                                                  s_f[:])

                        def emit_logZ(statg_ap, nj, j0):
                            """Combine gathered per-core expsums -> nlogZ
                            ([64, 1] f32, = -logZ)."""
                            sview = statg_ap.rearrange("p (c j) -> p j c",
                                                       j=nj)
                            S_t = work.tile([64, 1], FP32, tag="S_t",
                                            name="S_t", bufs=2)
                            nc.vector.tensor_reduce(
                                S_t[:], sview[:, j0 + 1:j0 + 2, :],
                                axis=AX.X, op=ALU.add)
                            lnS = work.tile([64, 1], FP32, tag="lnS",
                                            name="lnS", bufs=2)
                            nc.scalar.activation(lnS[:], S_t[:], AF.Ln)
                            nlogZ = work.tile([64, 1], FP32, tag="nlogZ",
                                              name="nlogZ", bufs=2)
                            nc.vector.tensor_scalar(nlogZ[:], lnS[:], -1.0,
                                                    None, op0=ALU.mult)
                            return nlogZ

                        def emit_out(s):
                            """Subtract logZ for block s and DMA out."""
                            lp_t = work.tile([64, VP], FP32, tag="lp_t",
                                             name="lp_t", bufs=2)
                            nc.vector.tensor_scalar(
                                lp_t[:], lg_sbs[s][:], nlogZs[s][:], None,
                                op0=ALU.add)
                            nc.sync.dma_start(
                                out=out_d[s * B:(s + 1) * B, :], in_=lp_t[:])
                            del lg_sbs[s], nlogZs[s]

                        for t in range(t_steps):
                            # ======== segment A: attention ========
                            agAx = work.tile([128, AGW], BF16, tag="agAx",
                                             name="agAx", bufs=2)
                            if t >= 2:
                                nc.vector.tensor_copy(
                                    agAx[0:64, 128:130],
                                    statb[:, ((t - 2) % 2) * 2:
                                          ((t - 2) % 2) * 2 + 2])
                            else:
                                nc.vector.memset(agAx[:, 128:130], 0.0)

                            if t >= 1:
                                ah_ps = psum.tile([B, H], FP32, tag="mid",
                                                  name="ah_ps", bufs=1)
                                for rc in range(RCN):
                                    nc.tensor.matmul(
                                        ah_ps[:],
                                        hT[:, rc * 64:(rc + 1) * 64],
                                        h2attT_s[rc][:], start=(rc == 0),
                                        stop=(rc == RCN - 1))
                                ah_sb = work.tile([B, H], BF16, tag="ah_sb",
                                                  name="ah_sb", bufs=1)
                                nc.vector.tensor_copy(ah_sb[:], ah_ps[:])
                                ahT_ps = psum.tile([128, HCN * 8], FP32,
                                                   tag="small",
                                                   name="ahT_ps", bufs=1)
                                for hc in range(HCN):
                                    nc.tensor.matmul(
                                        ahT_ps[:, hc * 8:(hc + 1) * 8],
                                        ah_sb[:, hc * 128:(hc + 1) * 128],
                                        bsel_s[:], start=True, stop=True)
                                ahT = work.tile([128, HCN * 8], BF16,
                                                tag="ahT_sb", name="ahT_sb",
                                                bufs=1)
                                nc.vector.tensor_copy(ahT[:], ahT_ps[:])

                            e_ps = psum.tile([BMY, L], FP32, tag="small",
                                             name="e_ps", bufs=1)
                            HB = BMY // 2
                            for hc in range(HCN):
                                if t >= 1:
                                    dp = work.tile([128, NBL], BF16,
                                                   tag="dp", name="dp",
                                                   bufs=2)
                                dt_ = work.tile([128, NBL], BF16, tag="dt",
                                                name="dt", bufs=2)
                                for bh in range(2):
                                    c0, c1 = bh * HB * L, (bh + 1) * HB * L
                                    if t == 0:
                                        # h0 == 0 -> dot = tanh(p_att)
                                        nc.scalar.activation(
                                            dt_[:, c0:c1],
                                            p_attT[hc][:, c0:c1], AF.Tanh)
                                    else:
                                        nc.vector.tensor_tensor(
                                            dp[:, c0:c1].rearrange(
                                                "p (b l) -> p b l", b=HB),
                                            p_attT[hc][:, c0:c1].rearrange(
                                                "p (b l) -> p b l", b=HB),
                                            bcast_free(
                                                ahT[:, hc * 8 + bh * HB:
                                                    hc * 8 + (bh + 1) * HB],
                                                L),
                                            op=ALU.add)
                                        nc.scalar.activation(dt_[:, c0:c1],
                                                             dp[:, c0:c1],
                                                             AF.Tanh)
                                    for b in range(bh * HB, (bh + 1) * HB):
                                        nc.tensor.matmul(
                                            e_ps[:],
                                            alpha_s[hc][:,
                                                        b * 8:(b + 1) * 8],
                                            dt_[:, b * L:(b + 1) * L],
                                            start=(hc == 0 and b == 0),
                                            stop=(hc == HCN - 1 and
                                                  b == BMY - 1))

                            # |e| <= ||alpha||_1 ~ 8, so exp(e) is far from
                            # f32/bf16 overflow: skip the max-shift and
                            # write bf16 directly into the scatter source
                            nc.scalar.activation(w_bf[:, 0:L], e_ps[:],
                                                 AF.Exp)

                            # scatter unnormalized weights into the
                            # (lp, b)-diagonal layout via a DRAM round trip
                            # (reads split across the SP and ACT dma
                            # queues); 1/sum folds into the psum copy below
                            wdr = dpool.tile([BMY, LP], BF16, tag="wdr",
                                             name="wdr")
                            nc.sync.dma_start(out=wdr[:], in_=w_bf[:])
                            for b in range(BMY):
                                dmae = nc.sync if b % 2 == 0 else nc.scalar
                                dmae.dma_start(
                                    out=stat_all[b:128:8, b:NG * 8:8],
                                    in_=wdr[b:b + 1, :].rearrange(
                                        "o (g lp) -> (o lp) g", g=NG))

                            ssum = work.tile([BMY, 1], FP32, tag="ssum",
                                             name="ssum", bufs=1)
                            nc.vector.tensor_reduce(ssum[:], w_bf[:, 0:L],
                                                    axis=AX.X, op=ALU.add)
                            rinv = work.tile([BMY, 1], FP32, tag="rinv",
                                             name="rinv", bufs=1)
                            nc.vector.reciprocal(rinv[:], ssum[:])

                            ar_sb = work.tile([BMY, F], BF16, tag="ar_sb",
                                              name="ar_sb", bufs=1)
                            for half in range(2):
                                f0 = half * 1024
                                ar_ps = psum.tile([BMY, 1024], FP32,
                                                  tag="ar", name="ar_ps",
                                                  bufs=1)
                                for g in range(NG):
                                    for qf in range(2):
                                        nc.tensor.matmul(
                                            ar_ps[:,
                                                  qf * 512:(qf + 1) * 512],
                                            stat_all[:, g * 8:(g + 1) * 8],
                                            att_lb_s[g][:,
                                                        f0 + qf * 512:
                                                        f0 + (qf + 1) * 512],
                                            start=(g == 0),
                                            stop=(g == NG - 1))
                                if half == 0:
                                    nc.vector.tensor_scalar(
                                        ar_sb[:, f0:f0 + 1024], ar_ps[:],
                                        rinv[:], None, op0=ALU.mult)
                                else:
                                    nc.scalar.activation(
                                        ar_sb[:, f0:f0 + 1024], ar_ps[:],
                                        AF.Copy, scale=rinv[:])

                            # transpose own att_res before the AllGather;
                            # stage straight from PSUM + statb to DRAM
                            arTo_ps = psum.tile([128, 128], BF16, tag="mid",
                                                name="arTo_ps", bufs=1)
                            for fc in range(FCN):
                                nc.tensor.transpose(
                                    arTo_ps[:, fc * 8:(fc + 1) * 8],
                                    ar_sb[:, fc * 128:(fc + 1) * 128],
                                    ident_s[0:BMY, 0:BMY])
                            nc.vector.tensor_copy(agAx[:, 0:128],
                                                  arTo_ps[:])
                            agA_in = dpool.tile([128, AGW], BF16,
                                                tag="agA_in", name="agA_in")
                            nc.sync.dma_start(out=agA_in[:], in_=agAx[:])
                            if no_cc:
                                nc.sync.dma_start(out=agA_out[t][0:128, :],
                                                  in_=agA_in[:])
                            else:
                                nc.gpsimd.collective_compute(
                                    "AllGather", ALU.bypass,
                                    replica_groups=RG,
                                    ins=[agA_in.opt()], outs=[agA_out[t][:]])

                            # ======== window 1 (during AG_A) ========
                            sums_ps = psum.tile([B, NGATE], FP32, tag="sums",
                                                name="sums", bufs=1)
                            for c0 in (0, 512):
                                c1 = min(NGATE, c0 + 512)
                                for kc in range(3):
                                    nc.tensor.matmul(
                                        sums_ps[:, c0:c1],
                                        xtT_s[kc][:, t * B:(t + 1) * B],
                                        i2hT_s[kc][:, c0:c1],
                                        start=(kc == 0),
                                        stop=(t == 0 and kc == 2))
                                if t >= 1:
                                    for rc in range(RCN):
                                        nc.tensor.matmul(
                                            sums_ps[:, c0:c1],
                                            hT[:, rc * 64:(rc + 1) * 64],
                                            h2hT_s[rc][:, c0:c1],
                                            start=False,
                                            stop=(rc == RCN - 1))
                            sig3 = work.tile([B, 384], FP32, tag="sig3",
                                             name="sig3", bufs=1)
                            nc.scalar.activation(sig3[:], sums_ps[:, 0:384],
                                                 AF.Tanh, scale=0.5)
                            sitr = work.tile([B, 256], FP32, tag="sitr",
                                             name="sitr", bufs=1)
                            nc.vector.tensor_copy(sitr[:],
                                                  sums_ps[:, 384:640])

                            if t >= 1:
                                s = t - 1
                                lg_sbs[s] = work.tile([64, VP], FP32,
                                                      tag="lg_sb",
                                                      name=f"lg_sb{s}",
                                                      bufs=3)
                                gsbA = work.tile([1, 1], BF16, tag="gsbA",
                                                 name="gsbA", bufs=2)
                                nc.sync.dma_start(out=gsbA[:],
                                                  in_=agA_in[0:1, 0:1])
                                logit_mms(s, 0, 512)
                                logit_mms(s, 512, 1024, gate=gsbA[:])

                            # ======== post-AG_A ========
                            # arTc cols: c*128 + fc*8 + b (one DMA),
                            # then DVE repack to fc-major for the matmul
                            arTc = work.tile([128, FCN * 64], BF16,
                                             tag="arTc", name="arTc",
                                             bufs=1)
                            _ag = agA_out[t][:]
                            arT_src = AP(_ag.tensor, _ag.offset,
                                         [[AGW, 128], [128 * AGW, NC],
                                          [1, FCN * BMY]])
                            nc.sync.dma_start(
                                out=arTc[:].rearrange("p (c fb) -> p c fb",
                                                      c=NC),
                                in_=arT_src)
                            arT = work.tile([128, FCN * 64], BF16, tag="arT",
                                            name="arT", bufs=1)
                            nc.vector.tensor_copy(
                                arT[:].rearrange("p (fc c b) -> p fc c b",
                                                 fc=FCN, c=NC),
                                arTc[:].rearrange("p (c fc b) -> p fc c b",
                                                  c=NC, fc=FCN))

                            ctx_ps = psum.tile([B, 256], FP32, tag="mid",
                                               name="ctx_ps", bufs=1)
                            for fc in range(FCN):
                                nc.tensor.matmul(
                                    ctx_ps[:], arT[:, fc * 64:(fc + 1) * 64],
                                    a2cT_s[fc][:], start=(fc == 0),
                                    stop=(fc == FCN - 1))

                            itr1 = work.tile([B, GC], FP32, tag="itr1",
                                             name="itr1", bufs=1)
                            nc.vector.tensor_tensor(itr1[:], sitr[:, 0:128],
                                                    ctx_ps[:, 0:128],
                                                    op=ALU.add)
                            itr2 = work.tile([B, GC], FP32, tag="itr2",
                                             name="itr2", bufs=1)
                            nc.vector.tensor_tensor(itr2[:],
                                                    sitr[:, 128:256],
                                                    ctx_ps[:, 128:256],
                                                    op=ALU.add)
                            g_t = work.tile([B, GC], FP32, tag="g_t",
                                            name="g_t", bufs=1)
                            nc.vector.tensor_tensor(g_t[:], itr1[:],
                                                    itr2[:], op=ALU.max)
                            a_t = work.tile([B, GC], FP32, tag="a_t",
                                            name="a_t", bufs=1)
                            nc.vector.scalar_tensor_tensor(
                                a_t[:], sig3[:, 128:256], 1.0, c_st[:],
                                op0=ALU.add, op1=ALU.mult)
                            b_t = work.tile([B, GC], FP32, tag="b_t",
                                            name="b_t", bufs=1)
                            nc.vector.scalar_tensor_tensor(
                                b_t[:], sig3[:, 0:128], 1.0, g_t[:],
                                op0=ALU.add, op1=ALU.mult)
                            nc2_t = work.tile([B, GC], FP32, tag="nc2",
                                              name="nc2", bufs=1)
                            nc.vector.tensor_tensor(nc2_t[:], a_t[:],
                                                    b_t[:], op=ALU.add)
                            nc.vector.tensor_scalar(c_st[:], nc2_t[:], 0.5,
                                                    None, op0=ALU.mult)
                            tnc = work.tile([B, GC], FP32, tag="tnc",
                                            name="tnc", bufs=1)
                            nc.scalar.activation(tnc[:], nc2_t[:], AF.Tanh,
                                                 scale=0.5)
                            nh2 = work.tile([B, GC], BF16, tag="nh2",
                                            name="nh2", bufs=1)
                            nc.vector.scalar_tensor_tensor(
                                nh2[:], sig3[:, 256:384], 1.0, tnc[:],
                                op0=ALU.add, op1=ALU.mult)

                            nhT_ps = psum.tile([GC, B], BF16, tag="small",
                                               name="nhT_ps", bufs=1)
                            nc.tensor.transpose(nhT_ps[:], nh2[:],
                                                ident_s[0:B, 0:B])
                            nhT_sb = work.tile([GC, B], BF16, tag="nhT_sb",
                                               name="nhT_sb", bufs=1)
                            nc.vector.tensor_copy(nhT_sb[:], nhT_ps[:])
                            agH_in = dpool.tile([GC, B], BF16, tag="agH_in",
                                                name="agH_in")
                            nc.sync.dma_start(out=agH_in[:], in_=nhT_sb[:])
                            if no_cc:
                                nc.sync.dma_start(out=agH_out[t][0:GC, :],
                                                  in_=agH_in[:])
                            else:
                                nc.gpsimd.collective_compute(
                                    "AllGather", ALU.bypass,
                                    replica_groups=RG,
                                    ins=[agH_in.opt()], outs=[agH_out[t][:]])

                            # ======== window 2 (during AG_H) ========
                            gH = agH_in[0:1, 0:1]
                            gsbH = work.tile([1, 1], BF16, tag="gsbH",
                                             name="gsbH", bufs=2)
                            nc.sync.dma_start(out=gsbH[:], in_=gH)
                            if t >= 1:
                                s = t - 1
                                logit_mms(s, 1024, VP, gate=gsbH[:])
                                logit_stats(s)
                            if t >= 2:
                                statg = work.tile([64, 2 * NC], BF16,
                                                  tag="statg", name="statg",
                                                  bufs=2)
                                nc.sync.dma_start(out=statg[0:1, 0:1],
                                                  in_=gH)
                                statg_src = AP(_ag.tensor, _ag.offset + 128,
                                               [[AGW, 64], [128 * AGW, NC],
                                                [1, 2]])
                                nc.sync.dma_start(
                                    out=statg[:].rearrange(
                                        "p (c j) -> p c j", c=NC),
                                    in_=statg_src)
                                nlogZs[t - 2] = emit_logZ(statg[:], 2, 0)
                                emit_out(t - 2)
                                # prefetch the tanh act-table set while the
                                # AG still runs (Ln swapped the set out)
                                dumt = work.tile([1, 1], FP32, tag="dumt",
                                                 name="dumt", bufs=1)
                                nc.scalar.activation(dumt[:],
                                                     statb[0:1, 0:1],
                                                     AF.Tanh)

                            # ======== post-AG_H ========
                            hT_new = hpool.tile([128, RCN * 64], BF16,
                                                tag="hT", name="hT_new")
                            nc.sync.dma_start(
                                out=hT_new[:].rearrange(
                                    "rl (rc b) -> rl rc b", rc=RCN),
                                in_=agH_out[t][:].rearrange(
                                    "(rc rl) b -> rl rc b", rc=RCN))
                            hT_hist.append(hT_new)
                            hT = hT_new

                            if t == 0:
                                probe("w0", w_bf[:], [BMY, LP], BF16)
                                probe("statall0", stat_all[:],
                                      [128, NG * 8], BF16)
                                probe("ar0", ar_sb[:], [BMY, F], BF16)
                                probe("nh20", nh2[:], [B, GC], BF16)
                                probe("hT1", hT_new[:], [128, RCN * 64],
                                      BF16)
                            if t == 1:
                                probe("lg0", lg_sbs[0][:], [64, VP], FP32)
                                probe("arT1", arT[:], [128, FCN * 64], BF16)

                        # ======== tail ========
                        s = t_steps - 1
                        lg_sbs[s] = work.tile([64, VP], FP32, tag="lg_sb",
                                              name=f"lg_sb{s}", bufs=3)
                        logit_mms(s, 0, 512)
                        logit_mms(s, 512, 1024)
                        logit_mms(s, 1024, VP)
                        logit_stats(s)

                        agS_in = dpool.tile([64, 4], BF16, tag="agS_in",
                                            name="agS_in")
                        nc.sync.dma_start(out=agS_in[:], in_=statb[:])
                        if no_cc:
                            nc.sync.dma_start(out=agS_out[0:64, :],
                                              in_=agS_in[:])
                        else:
                            nc.gpsimd.collective_compute(
                                "AllGather", ALU.bypass, replica_groups=RG,
                                ins=[agS_in.opt()], outs=[agS_out[:]])
                        statg2 = work.tile([64, 4 * NC], BF16, tag="statg2",
                                           name="statg2", bufs=1)
                        _ags = agS_out[:]
                        statg2_src = AP(_ags.tensor, _ags.offset,
                                        [[4, 64], [64 * 4, NC], [1, 4]])
                        nc.sync.dma_start(out=statg2[:], in_=statg2_src)
                        for s in (t_steps - 2, t_steps - 1):
                            nlogZs[s] = emit_logZ(statg2[:], 4, (s % 2) * 2)
                            emit_out(s)

                for rep in range(reps):
                    emit_rep(rep)

    nc.compile()
    return nc, sorted(probes)


_NC_CACHE = {}


def kernel(**inputs):
    """Full-input entry point: returns logp [B, T, V1] float32."""
    from concourse.bass_utils import run_bass_kernel_spmd
    in_maps = host_prep(inputs)
    if "nc" not in _NC_CACHE:
        _NC_CACHE["nc"], _ = build(T, (), reps=1)
    nc = _NC_CACHE["nc"]
    res = run_bass_kernel_spmd(nc, in_maps, list(range(NC)))
    outs = [res.results[c]["logp"] for c in range(NC)]
    full = np.concatenate(outs, axis=1)[:, :V1]          # [T*B, V1]
    logp = full.reshape(T, B, V1).transpose(1, 0, 2)
    return np.ascontiguousarray(logp.astype(np.float32))
